# revision 21
# baseline (speedup 1.0000x reference)
"""Trainium2 Bass kernel for nn_CIFAR_SSM_Classifier.

Data-parallel over 8 NeuronCores: each core processes 64 of the 512 images.

Per-core pipeline (SBUF-resident, bf16 matmuls on the PE, fp32 accumulate).
All conv matmuls use the full K=128 contraction rows (partial-K matmuls let
the PE activity monitor drop the clock to 1.2 GHz):
  conv1 (3->64)   : 9 taps x 3ch stacked on K (27 rows, zero-padded to 128);
                    the tap-shifted/zero-padded input layout is staged host-side
                    and DMA'd once per sub-batch.
  conv2 (64->128) : 5 K=128 matmuls per 512-col bank: 3 pairs {(dy,-1),(dy,0)}
                    via fm1 upper half = fm1 shifted +1 col; 1 pair
                    {(-1,+1),(0,+1)} via scratch tile (lower=fm1, upper=fm1
                    shifted +34); 1 single (1,+1) with zero-padded weights.
  conv3 (128->128): 9 K=128 matmuls per bank.
  BN scale folded into conv weights host-side; drains are relu(psum+beta),
  split between the Vector and Scalar engines.
  Feature maps use a zero-padded 34x34 frame layout per image so all taps are
  plain column offsets of one SBUF tile.
  width-mean -> SSM: the L=32 diagonal scan is unrolled algebraically:
    sum_t x_t = sum_tau w_tau (.) (B u_tau),  w_tau = sum_{k<=L-1-tau} A^k
  and the Bu matmul + weighted tau-reduction run incrementally per sub-batch.
"""
import numpy as np
import ml_dtypes

import concourse.bass as bass
import concourse.tile as tile
from concourse import bacc, mybir
from concourse.bass_utils import run_bass_kernel_spmd
from concourse.masks import make_identity

F32 = mybir.dt.float32
BF16 = mybir.dt.bfloat16
AF = mybir.ActivationFunctionType
ALU = mybir.AluOpType

NCORES = 8
B = 512
BL = B // NCORES          # 64 images per core
NI = 16                   # images per sub-batch
SUB = BL // NI            # 4 sub-batches
FR = 34 * 34              # padded frame (34x34) per image
SPAN = NI * FR            # 18496
G = 72                    # guard columns on each side
W = G + SPAN + G
TAPS = [(dy, dx) for dy in (-1, 0, 1) for dx in (-1, 0, 1)]
PSPAN = NI * 1024          # packed interior span (matmul out / psum domain)
CH = [(c, 1024) for c in range(0, PSPAN, 1024)]
L = 32
S = 256


def _banks(length):
    return [(b, min(512, length - b)) for b in range(0, length, 512)]


def _rhs(src, p0, p1, bank, d):
    # interior pixels of half-frame `bank` (frame bank//2, rows 16*(bank%2)..+16)
    # of the padded layout, shifted by tap offset d
    n, hh = bank // 2, bank % 2
    base = G + n * FR + 35 + 544 * hh + d
    return src[p0:p1, base:base + 16 * 34].rearrange(
        "p (h w) -> p h w", w=34)[:, :, 0:32]


def _interior2(dst, p0, p1, c0):
    # interior pixels of frame c0//1024 of the padded layout (3-D view)
    n = c0 // 1024
    base = G + n * FR + 35
    return dst[p0:p1, base:base + 32 * 34].rearrange(
        "p (h w) -> p h w", w=34)[:, :, 0:32]


def build():
    nc = bacc.Bacc(None, target_bir_lowering=False, debug=False)

    x_d = nc.declare_dram_parameter("xst", [SUB, 32, W], BF16, isOutput=False)
    c1_d = nc.declare_dram_parameter("c1T", [128, 64], BF16, isOutput=False)
    c2q_d = nc.declare_dram_parameter("c2q", [5, 128, 128], BF16, isOutput=False)
    c3_d = nc.declare_dram_parameter("c3T", [9, 128, 128], BF16, isOutput=False)
    sc_d = {}
    for i, cc in ((1, 64), (2, 128), (3, 128)):
        sc_d[i] = nc.declare_dram_parameter(f"beta{i}", [cc], F32, isOutput=False)
    bt_d = nc.declare_dram_parameter("BT", [128, S], BF16, isOutput=False)
    wt_d = nc.declare_dram_parameter("Wt", [128, 2, L], F32, isOutput=False)
    ct_d = nc.declare_dram_parameter("Ct", [2, 128, S], F32, isOutput=False)
    dt_d = nc.declare_dram_parameter("Dt", [128, S], F32, isOutput=False)
    w1_d = nc.declare_dram_parameter("w1T", [2, 128, 128], F32, isOutput=False)
    w2_d = nc.declare_dram_parameter("w2T", [128, 10], F32, isOutput=False)
    b1_d = nc.declare_dram_parameter("hb1", [128], F32, isOutput=False)
    b2_d = nc.declare_dram_parameter("hb2", [10], F32, isOutput=False)
    pb_d = nc.declare_dram_parameter("pbias", [128, 2], F32, isOutput=False)
    out1_d = nc.declare_dram_parameter("out1", [BL, 10], F32, isOutput=True)
    out2_d = nc.declare_dram_parameter("out2", [BL, S], F32, isOutput=True)

    with tile.TileContext(nc) as tc:
        import contextlib
        with contextlib.ExitStack() as ctx:
            consts = ctx.enter_context(tc.tile_pool(name="consts", bufs=1))
            big = ctx.enter_context(tc.tile_pool(name="big", bufs=1))

            # ---- tiles
            c1w = consts.tile([128, 64], BF16)
            c2w = consts.tile([128, 5, 128], BF16)
            c3w = consts.tile([128, 9, 128], BF16)
            btw = consts.tile([128, S], BF16)
            wtw = consts.tile([128, 2, L], F32)
            ctw = consts.tile([128, 2, S], F32)
            dtw = consts.tile([128, S], F32)
            w1w = consts.tile([128, 2, 128], F32)
            w2w = consts.tile([128, 10], F32)
            b1w = consts.tile([128, 1], F32)
            b2w = consts.tile([16, 1], F32)
            pbw = consts.tile([128, 2], F32)
            ident = consts.tile([128, 128], F32)
            sc = {}
            for i, cc in ((1, 64), (2, 128), (3, 128)):
                sc[i] = consts.tile([cc, 1], F32, tag=f"beta{i}", name=f"beta{i}")

            x_st = big.tile([128, W], BF16)    # 0-26: staged taps, 27-127 zero
            fm1 = big.tile([128, W], BF16)     # 0-63: conv1 out; 64-127: +1 col
            fm2 = big.tile([128, W], BF16)
            scr = big.tile([128, W], BF16)     # conv2: fm1b (+0/+34); conv3: fm3
            u = big.tile([128, BL, L], BF16)   # width-sums, all 64 images
            sx = [big.tile([128, BL], F32, tag=f"sx{m}", name=f"sx{m}")
                  for m in range(2)]
            ub = big.tile([128, BL], F32)

            # ---- startup-critical loads; x[0] split across all 3 DMA rings
            # with column ranges ordered so conv1 consumes them just-in-time
            nc.sync.dma_start(c1w[:], c1_d[:, :])
            for i in (1, 2, 3):
                nc.sync.dma_start(sc[i][:], sc_d[i][:].unsqueeze(1))
            nc.scalar.dma_start(out=x_st[0:32, 4096:11264],
                                in_=x_d[0, :, 4096:11264])
            nc.gpsimd.dma_start(out=x_st[0:32, 11264:W],
                                in_=x_d[0, :, 11264:W])
            nc.sync.dma_start(out=x_st[0:32, 0:4096], in_=x_d[0, :, 0:4096])
            nc.sync.dma_start(c2w[:], c2q_d[:, :, :].rearrange("t k m -> k t m"))
            nc.sync.dma_start(c3w[:], c3_d[:, :, :].rearrange("t k m -> k t m"))
            # SSM weights used from sub-batch 0's tail
            nc.gpsimd.dma_start(btw[:], bt_d[:, :])
            nc.gpsimd.dma_start(wtw[:], wt_d[:, :, :])

            # minimal zero-init: x_st rows 27-127 (read with zero weights) and
            # the guard columns of the feature maps

            for t in (fm1, fm2, scr):
                nc.vector.memset(t[:, 0:G], 0.0)
                nc.vector.memset(t[:, G + SPAN:W], 0.0)

            rings_once = True

            def rings(t, p1, engine):  # noqa: E306
                for j in range(NI):
                    F0 = G + j * FR
                    engine.memset(t[0:p1, F0:F0 + 35], 0.0)
                    rb = t[0:p1, F0 + 67:F0 + 67 + 31 * 34].rearrange(
                        "p (a b) -> p a b", b=34)[:, :, 0:2]
                    engine.memset(rb, 0.0)
                    engine.memset(t[0:p1, F0 + 1121:F0 + 1156], 0.0)

            rings(fm1, 64, nc.vector)
            nc.vector.memset(x_st[32:64, :], 0.0)
            nc.scalar.memzero(x_st[64:96, :])
            nc.gpsimd.memset(x_st[96:128, :], 0.0)


            with tc.tile_pool(name="cps", bufs=2, space="PSUM") as cps:
                for k in range(SUB):
                    b0 = k * NI
                    if k > 0:
                        nc.sync.dma_start(out=x_st[0:32, :], in_=x_d[k, :, :])

                    # ---- conv1 (K padded to 128; lhsT rows 27-127 are 0);
                    # drains alternate DVE / ACT
                    for ci, (c0, ln) in enumerate(CH):
                        pt = cps.tile([128, 1024], F32, tag="cps", bufs=4)
                        for (bo, bl) in _banks(ln):
                            nc.tensor.matmul(
                                pt[0:64, bo:bo + bl], c1w[:],
                                _rhs(x_st, 0, 128, (c0 + bo) // 512, 0),
                                start=True, stop=True)
                        if k > 0 and ci % 2 == 0:
                            with nc.allow_low_precision(reason="bf16 fm"):
                                nc.vector.tensor_scalar(
                                    _interior2(fm1, 0, 64, c0),
                                    pt[0:64, 0:ln].rearrange(
                                        "p (h w) -> p h w", w=32),
                                    sc[1][:], 0.0, op0=ALU.add, op1=ALU.max)
                        else:
                            nc.scalar.activation(
                                _interior2(fm1, 0, 64, c0),
                                pt[0:64, 0:ln].rearrange(
                                    "p (h w) -> p h w", w=32),
                                AF.Relu, bias=sc[1][:], scale=1.0)
                    # staging for conv2: fm1 upper = fm1+1; scr = [fm1; fm1+34]
                    # (padded-span chunks, not packed ones)
                    for c0 in range(0, SPAN, 2048):
                        ln = min(2048, SPAN - c0)
                        a = G + c0
                        nc.sync.dma_start(out=fm1[64:128, a:a + ln],
                                          in_=fm1[0:64, a + 1:a + ln + 1])
                        nc.gpsimd.dma_start(out=scr[0:64, a:a + ln],
                                            in_=fm1[0:64, a:a + ln])
                        nc.gpsimd.dma_start(out=scr[64:128, a:a + ln],
                                            in_=fm1[0:64, a + 34:a + ln + 34])

                    # ---- conv2: 5 x K=128 per bank
                    for (c0, ln) in CH:
                        pt = cps.tile([128, 1024], F32, tag="cps", bufs=4)
                        for (bo, bl) in _banks(ln):
                            bank = (c0 + bo) // 512
                            mms = [(0, fm1, -35), (1, fm1, -1), (2, fm1, 33),
                                   (3, scr, -33), (4, scr, 35)]
                            for qi, (q, src, d) in enumerate(mms):
                                nc.tensor.matmul(
                                    pt[:, bo:bo + bl], c2w[:, q, :],
                                    _rhs(src, 0, 128, bank, d),
                                    start=(qi == 0), stop=(qi == 4))
                        if (c0 // 1024) % 2 == 0:
                            with nc.allow_low_precision(reason="bf16 fm"):
                                nc.vector.tensor_scalar(
                                    _interior2(fm2, 0, 128, c0),
                                    pt[:, 0:ln].rearrange("p (h w) -> p h w", w=32),
                                    sc[2][:], 0.0, op0=ALU.add, op1=ALU.max)
                        else:
                            nc.scalar.activation(
                                _interior2(fm2, 0, 128, c0),
                                pt[:, 0:ln].rearrange("p (h w) -> p h w", w=32),
                                AF.Relu, bias=sc[2][:], scale=1.0)

                    if k == 0:
                        rings(fm2, 128, nc.vector)
                    # ---- conv3 (fm3 lives in scr; conv2 reads of scr are done)
                    for (c0, ln) in CH:
                        pt = cps.tile([128, 1024], F32, tag="cps", bufs=4)
                        for (bo, bl) in _banks(ln):
                            bank = (c0 + bo) // 512
                            for t, (dy, dx) in enumerate(TAPS):
                                d = 34 * dy + dx
                                nc.tensor.matmul(
                                    pt[:, bo:bo + bl], c3w[:, t, :],
                                    _rhs(fm2, 0, 128, bank, d),
                                    start=(t == 0), stop=(t == 8))
                        if (c0 // 1024) % 2 == 1:
                            with nc.allow_low_precision(reason="bf16 fm"):
                                nc.vector.tensor_scalar(
                                    scr[:, G + c0:G + c0 + ln], pt[:, 0:ln],
                                    sc[3][:], 0.0, op0=ALU.add, op1=ALU.max)
                        else:
                            nc.scalar.activation(
                                scr[:, G + c0:G + c0 + ln], pt[:, 0:ln],
                                AF.Relu, bias=sc[3][:], scale=1.0)

                    # ---- width sums -> u (per frame, pipelined w/ conv3 drains)
                    for j in range(NI):
                        ivj = scr[:, G + j * 1024:G + (j + 1) * 1024].rearrange(
                            "p (h w) -> p h w", w=32)
                        with nc.allow_low_precision(reason="bf16 u; ~2e-3 ok"):
                            nc.vector.tensor_reduce(
                                u[:, b0 + j:b0 + j + 1, :], ivj,
                                axis=mybir.AxisListType.X, op=ALU.add)

                    # ---- incremental SSM for this sub-batch's 512 (b,tau) cols
                    ucols = u[:, b0:b0 + NI, :].rearrange("p a b -> p (a b)")
                    for m in range(2):
                        pm = cps.tile([128, 512], F32, tag="cps", bufs=4)
                        nc.tensor.matmul(pm[:], btw[:, 128 * m:128 * (m + 1)],
                                         ucols, start=True, stop=True)
                        tmp = big.tile([128, NI, L], F32, tag="tmp")
                        nc.vector.tensor_tensor(
                            tmp[:], pm[:].rearrange("p (a b) -> p a b", b=L),
                            wtw[:, m:m + 1, :].broadcast_to((128, NI, L)),
                            op=ALU.mult)
                        nc.vector.tensor_reduce(
                            sx[m][:, b0:b0 + NI], tmp[:],
                            axis=mybir.AxisListType.X, op=ALU.add)
                    with nc.allow_low_precision(reason="sum of bf16 u, f32 out"):
                        nc.vector.tensor_reduce(
                            ub[:, b0:b0 + NI], u[:, b0:b0 + NI, :],
                            axis=mybir.AxisListType.X, op=ALU.add)

                # ---- tail-only constants (emitted late: sync ring is idle now)
                nc.sync.dma_start(ctw[:], ct_d[:, :, :].rearrange("k p o -> p k o"))
                nc.sync.dma_start(dtw[:], dt_d[:, :])
                nc.sync.dma_start(w1w[:], w1_d[:, :, :].rearrange("m p o -> p m o"))
                nc.sync.dma_start(w2w[:], w2_d[:, :])
                nc.sync.dma_start(b1w[:], b1_d[:].unsqueeze(1))
                nc.sync.dma_start(b2w[0:10, :], b2_d[:].unsqueeze(1))
                nc.sync.dma_start(pbw[:], pb_d[:, :])
                make_identity(nc, ident)

            with tc.tile_pool(name="tail", bufs=1, space="PSUM") as tps:
                # pooled[o,b] = Ct.T@sx0 + Ct.T@sx1 + Dt.T@ub  (+ h0 bias)
                pooled_s = []
                o2s = big.tile([64, S], F32)
                for m in range(2):
                    pp = tps.tile([128, BL], F32, tag=f"pl{m}")
                    ops = [(ctw[:, 0, 128 * m:128 * (m + 1)], sx[0]),
                           (ctw[:, 1, 128 * m:128 * (m + 1)], sx[1]),
                           (dtw[:, 128 * m:128 * (m + 1)], ub)]
                    for i, (lt_, rt) in enumerate(ops):
                        nc.tensor.matmul(pp[:], lt_, rt[:],
                                         start=(i == 0), stop=(i == 2))
                    ps_t = big.tile([128, BL], F32, tag=f"pooled{m}")
                    nc.scalar.activation(ps_t[:], pp[:], AF.Identity,
                                         bias=pbw[:, m:m + 1], scale=1.0)
                    pooled_s.append(ps_t)
                    ptr = tps.tile([64, 128], F32, tag="ptr", bufs=2)
                    nc.tensor.transpose(ptr[:], ps_t[:], ident[:])
                    nc.vector.tensor_copy(o2s[:, 128 * m:128 * (m + 1)], ptr[:])
                nc.sync.dma_start(out2_d[:, :], o2s[:])

                # head
                hp = tps.tile([128, BL], F32, tag="hp")
                for m in range(2):
                    nc.tensor.matmul(hp[:], w1w[:, m, :], pooled_s[m][:],
                                     start=(m == 0), stop=(m == 1))
                hs = big.tile([128, BL], F32)
                nc.scalar.activation(hs[:], hp[:], AF.Relu, bias=b1w[:], scale=1.0)
                lp = tps.tile([16, BL], F32, tag="lp")
                nc.tensor.matmul(lp[0:10, :], w2w[:], hs[:], start=True, stop=True)
                ls = big.tile([16, BL], F32)
                nc.scalar.activation(ls[0:10, :], lp[0:10, :], AF.Identity,
                                     bias=b2w[0:10, :], scale=1.0)
                lt = tps.tile([64, 16], F32, tag="lt")
                nc.tensor.transpose(lt[:, 0:10], ls[0:10, :], ident[0:10, 0:10])
                o1s = big.tile([64, 16], F32)
                nc.vector.tensor_copy(o1s[:, 0:10], lt[:, 0:10])
                nc.sync.dma_start(out1_d[:, :], o1s[:, 0:10])

    nc.finalize()
    return nc


def prep_in_maps(inputs):
    f32 = np.float32
    bf = ml_dtypes.bfloat16

    scb0 = {}
    beta = {}
    for i in (1, 2, 3):
        g = np.asarray(inputs[f"bn{i}_g"], f32)
        b = np.asarray(inputs[f"bn{i}_b"], f32)
        m = np.asarray(inputs[f"bn{i}_m"], f32)
        v = np.asarray(inputs[f"bn{i}_v"], f32)
        inv = g / np.sqrt(v + np.float32(1e-5))
        scb0[i] = inv
        beta[i] = (b - m * inv).astype(f32)
    c1 = np.asarray(inputs["conv1_w"], dtype=f32) * scb0[1][:, None, None, None]
    c1T = np.zeros((128, 64), f32)
    for t, (dy, dx) in enumerate(TAPS):
        c1T[3 * t:3 * t + 3, :] = c1[:, :, dy + 1, dx + 1].T
    c2 = np.asarray(inputs["conv2_w"], dtype=f32) * scb0[2][:, None, None, None]
    c2q = np.zeros((5, 128, 128), f32)
    for i, dy in enumerate((-1, 0, 1)):
        c2q[i, 0:64, :] = c2[:, :, dy + 1, 0].T
        c2q[i, 64:128, :] = c2[:, :, dy + 1, 1].T
    c2q[3, 0:64, :] = c2[:, :, 0, 2].T
    c2q[3, 64:128, :] = c2[:, :, 1, 2].T
    c2q[4, 0:64, :] = c2[:, :, 2, 2].T
    c3 = np.asarray(inputs["conv3_w"], dtype=f32) * scb0[3][:, None, None, None]
    c3T = np.zeros((9, 128, 128), f32)
    for t, (dy, dx) in enumerate(TAPS):
        c3T[t] = c3[:, :, dy + 1, dx + 1].T

    A = -np.log1p(np.exp(np.asarray(inputs["ssm_A"], np.float64)))
    wts = np.stack([(1.0 - A ** (L - t)) / (1.0 - A) for t in range(L)], 1)  # (S,L)
    Wt = (wts / (32.0 * L)).astype(f32).reshape(2, 128, L).transpose(1, 0, 2)
    Wt = np.ascontiguousarray(Wt)
    BT = np.ascontiguousarray(np.asarray(inputs["ssm_B"], f32).T)
    Cm = np.asarray(inputs["ssm_C"], f32)
    Ct = np.ascontiguousarray(Cm.T.reshape(2, 128, S))
    Dt = np.ascontiguousarray((np.asarray(inputs["ssm_D"], np.float64).T / (32.0 * L)).astype(f32))
    h0 = np.asarray(inputs["ssm_h0"], np.float64)
    geo = A * (1.0 - A ** L) / (1.0 - A)
    pbias = ((Cm.astype(np.float64) @ (geo * h0)) / L).astype(f32).reshape(2, 128).T
    pbias = np.ascontiguousarray(pbias)

    w1T = np.ascontiguousarray(np.asarray(inputs["head_w1"], f32).T.reshape(2, 128, 128))
    w2T = np.ascontiguousarray(np.asarray(inputs["head_w2"], f32).T)
    hb1 = np.asarray(inputs["head_b1"], f32)
    hb2 = np.asarray(inputs["head_b2"], f32)

    shared = dict(c1T=c1T.astype(bf), c2q=c2q.astype(bf), c3T=c3T.astype(bf),
                  beta1=beta[1], beta2=beta[2], beta3=beta[3],
                  BT=BT.astype(bf), Wt=Wt, Ct=Ct, Dt=Dt, w1T=w1T, w2T=w2T,
                  hb1=hb1, hb2=hb2, pbias=pbias)

    x = np.asarray(inputs["x"], f32)
    in_maps = []
    for i in range(NCORES):
        xc = x[i * BL:(i + 1) * BL].reshape(SUB, NI, 3, 32, 32)
        wide = np.zeros((SUB, 3, W + 70), f32)
        wv = wide[:, :, 35 + G:35 + G + NI * FR].reshape(SUB, 3, NI, 34, 34)
        wv[:, :, :, 1:33, 1:33] = xc.transpose(0, 2, 1, 3, 4)
        xst = np.zeros((SUB, 32, W), f32)
        for t, (dy, dx) in enumerate(TAPS):
            d = 34 * dy + dx
            xst[:, 3 * t:3 * t + 3, :] = wide[:, :, 35 + d:35 + d + W]
        m = dict(shared)
        m["xst"] = np.ascontiguousarray(xst.astype(bf))
        in_maps.append(m)
    return in_maps


_NC_CACHE = []


def kernel(**inputs):
    if not _NC_CACHE:
        _NC_CACHE.append(build())
    nc = _NC_CACHE[0]
    in_maps = prep_in_maps(inputs)
    res = run_bass_kernel_spmd(nc, in_maps, core_ids=list(range(NCORES)))
    out = np.concatenate([res.results[i]["out1"] for i in range(NCORES)], axis=0)
    act = np.concatenate([res.results[i]["out2"] for i in range(NCORES)], axis=0)
    return out.astype(np.float32), act.astype(np.float32)


# revision 22
# speedup vs baseline: 1.0298x; 1.0298x over previous
"""Trainium2 Bass kernel for nn_CIFAR_SSM_Classifier.

Data-parallel over 8 NeuronCores: each core processes 64 of the 512 images.

Per-core pipeline (SBUF-resident, bf16 matmuls on the PE, fp32 accumulate).
All conv matmuls use the full K=128 contraction rows (partial-K matmuls let
the PE activity monitor drop the clock to 1.2 GHz):
  conv1 (3->64)   : 9 taps x 3ch stacked on K (27 rows, zero-padded to 128);
                    the tap-shifted/zero-padded input layout is staged host-side
                    and DMA'd once per sub-batch.
  conv2 (64->128) : 5 K=128 matmuls per 512-col bank: 3 pairs {(dy,-1),(dy,0)}
                    via fm1 upper half = fm1 shifted +1 col; 1 pair
                    {(-1,+1),(0,+1)} via scratch tile (lower=fm1, upper=fm1
                    shifted +34); 1 single (1,+1) with zero-padded weights.
  conv3 (128->128): 9 K=128 matmuls per bank.
  BN scale folded into conv weights host-side; drains are relu(psum+beta),
  split between the Vector and Scalar engines.
  Feature maps use a zero-padded 34x34 frame layout per image so all taps are
  plain column offsets of one SBUF tile.
  width-mean -> SSM: the L=32 diagonal scan is unrolled algebraically:
    sum_t x_t = sum_tau w_tau (.) (B u_tau),  w_tau = sum_{k<=L-1-tau} A^k
  and the Bu matmul + weighted tau-reduction run incrementally per sub-batch.
"""
import numpy as np
import ml_dtypes

import concourse.bass as bass
import concourse.tile as tile
from concourse import bacc, mybir
from concourse.bass_utils import run_bass_kernel_spmd
from concourse.masks import make_identity

F32 = mybir.dt.float32
BF16 = mybir.dt.bfloat16
AF = mybir.ActivationFunctionType
ALU = mybir.AluOpType

NCORES = 8
B = 512
BL = B // NCORES          # 64 images per core
NI = 16                   # images per sub-batch
SUB = BL // NI            # 4 sub-batches
FR = 34 * 34              # padded frame (34x34) per image
SPAN = NI * FR            # 18496
G = 72                    # guard columns on each side
W = G + SPAN + G
TAPS = [(dy, dx) for dy in (-1, 0, 1) for dx in (-1, 0, 1)]
PSPAN = NI * 1024          # packed interior span (matmul out / psum domain)
CH = [(c, 1024) for c in range(0, PSPAN, 1024)]
L = 32
S = 256


def _banks(length):
    return [(b, min(512, length - b)) for b in range(0, length, 512)]


def _rhs(src, p0, p1, bank, d):
    # interior pixels of half-frame `bank` (frame bank//2, rows 16*(bank%2)..+16)
    # of the padded layout, shifted by tap offset d
    n, hh = bank // 2, bank % 2
    base = G + n * FR + 35 + 544 * hh + d
    return src[p0:p1, base:base + 16 * 34].rearrange(
        "p (h w) -> p h w", w=34)[:, :, 0:32]


def _interior2(dst, p0, p1, c0):
    # interior pixels of frame c0//1024 of the padded layout (3-D view)
    n = c0 // 1024
    base = G + n * FR + 35
    return dst[p0:p1, base:base + 32 * 34].rearrange(
        "p (h w) -> p h w", w=34)[:, :, 0:32]


def build():
    nc = bacc.Bacc(None, target_bir_lowering=False, debug=False)

    x_d = nc.declare_dram_parameter("xst", [SUB, 32, W], BF16, isOutput=False)
    c1_d = nc.declare_dram_parameter("c1T", [32, 64], BF16, isOutput=False)
    c2q_d = nc.declare_dram_parameter("c2q", [5, 128, 128], BF16, isOutput=False)
    c3_d = nc.declare_dram_parameter("c3T", [9, 128, 128], BF16, isOutput=False)
    sc_d = {}
    for i, cc in ((1, 64), (2, 128), (3, 128)):
        sc_d[i] = nc.declare_dram_parameter(f"beta{i}", [cc], F32, isOutput=False)
    bt_d = nc.declare_dram_parameter("BT", [128, S], BF16, isOutput=False)
    wt_d = nc.declare_dram_parameter("Wt", [128, 2, L], F32, isOutput=False)
    ct_d = nc.declare_dram_parameter("Ct", [2, 128, S], F32, isOutput=False)
    dt_d = nc.declare_dram_parameter("Dt", [128, S], F32, isOutput=False)
    w1_d = nc.declare_dram_parameter("w1T", [2, 128, 128], F32, isOutput=False)
    w2_d = nc.declare_dram_parameter("w2T", [128, 10], F32, isOutput=False)
    b1_d = nc.declare_dram_parameter("hb1", [128], F32, isOutput=False)
    b2_d = nc.declare_dram_parameter("hb2", [10], F32, isOutput=False)
    pb_d = nc.declare_dram_parameter("pbias", [128, 2], F32, isOutput=False)
    out1_d = nc.declare_dram_parameter("out1", [BL, 10], F32, isOutput=True)
    out2_d = nc.declare_dram_parameter("out2", [BL, S], F32, isOutput=True)

    with tile.TileContext(nc) as tc:
        import contextlib
        with contextlib.ExitStack() as ctx:
            consts = ctx.enter_context(tc.tile_pool(name="consts", bufs=1))
            big = ctx.enter_context(tc.tile_pool(name="big", bufs=1))

            # ---- tiles
            c1w = consts.tile([32, 64], BF16)
            c2w = consts.tile([128, 5, 128], BF16)
            c3w = consts.tile([128, 9, 128], BF16)
            btw = consts.tile([128, S], BF16)
            wtw = consts.tile([128, 2, L], F32)
            ctw = consts.tile([128, 2, S], F32)
            dtw = consts.tile([128, S], F32)
            w1w = consts.tile([128, 2, 128], F32)
            w2w = consts.tile([128, 10], F32)
            b1w = consts.tile([128, 1], F32)
            b2w = consts.tile([16, 1], F32)
            pbw = consts.tile([128, 2], F32)
            ident = consts.tile([128, 128], F32)
            sc = {}
            for i, cc in ((1, 64), (2, 128), (3, 128)):
                sc[i] = consts.tile([cc, 1], F32, tag=f"beta{i}", name=f"beta{i}")

            x_st = big.tile([32, W], BF16)     # 27 stacked shifted taps of x
            fm1 = big.tile([128, W], BF16)     # 0-63: conv1 out; 64-127: +1 col
            fm2 = big.tile([128, W], BF16)
            scr = big.tile([128, W], BF16)     # conv2: fm1b (+0/+34); conv3: fm3
            u = big.tile([128, BL, L], BF16)   # width-sums, all 64 images
            sx = [big.tile([128, BL], F32, tag=f"sx{m}", name=f"sx{m}")
                  for m in range(2)]
            ub = big.tile([128, BL], F32)

            # ---- startup-critical loads; x[0] split across all 3 DMA rings
            # with column ranges ordered so conv1 consumes them just-in-time
            nc.sync.dma_start(c1w[:], c1_d[:, :])
            for i in (1, 2, 3):
                nc.sync.dma_start(sc[i][:], sc_d[i][:].unsqueeze(1))
            nc.scalar.dma_start(out=x_st[0:32, 4096:11264],
                                in_=x_d[0, :, 4096:11264])
            nc.gpsimd.dma_start(out=x_st[0:32, 11264:W],
                                in_=x_d[0, :, 11264:W])
            nc.sync.dma_start(out=x_st[0:32, 0:4096], in_=x_d[0, :, 0:4096])
            nc.sync.dma_start(c2w[:], c2q_d[:, :, :].rearrange("t k m -> k t m"))
            nc.sync.dma_start(c3w[:], c3_d[:, :, :].rearrange("t k m -> k t m"))
            # SSM weights used from sub-batch 0's tail
            nc.gpsimd.dma_start(btw[:], bt_d[:, :])
            nc.gpsimd.dma_start(wtw[:], wt_d[:, :, :])

            # minimal zero-init: x_st rows 27-127 (read with zero weights) and
            # the guard columns of the feature maps

            for t in (fm1, fm2, scr):
                nc.vector.memset(t[:, 0:G], 0.0)
                nc.vector.memset(t[:, G + SPAN:W], 0.0)

            rings_once = True

            def rings(t, p1, engine):  # noqa: E306
                for j in range(NI):
                    F0 = G + j * FR
                    engine.memset(t[0:p1, F0:F0 + 35], 0.0)
                    rb = t[0:p1, F0 + 67:F0 + 67 + 31 * 34].rearrange(
                        "p (a b) -> p a b", b=34)[:, :, 0:2]
                    engine.memset(rb, 0.0)
                    engine.memset(t[0:p1, F0 + 1121:F0 + 1156], 0.0)

            rings(fm1, 64, nc.vector)


            with tc.tile_pool(name="cps", bufs=2, space="PSUM") as cps:
                for k in range(SUB):
                    b0 = k * NI
                    if k > 0:
                        nc.sync.dma_start(out=x_st[0:32, :], in_=x_d[k, :, :])

                    # ---- conv1 (K padded to 128; lhsT rows 27-127 are 0);
                    # drains alternate DVE / ACT
                    for ci, (c0, ln) in enumerate(CH):
                        pt = cps.tile([128, 1024], F32, tag="cps", bufs=4)
                        for (bo, bl) in _banks(ln):
                            nc.tensor.matmul(
                                pt[0:64, bo:bo + bl], c1w[:],
                                _rhs(x_st, 0, 32, (c0 + bo) // 512, 0),
                                start=True, stop=True)
                        if k > 0 and ci % 2 == 0:
                            with nc.allow_low_precision(reason="bf16 fm"):
                                nc.vector.tensor_scalar(
                                    _interior2(fm1, 0, 64, c0),
                                    pt[0:64, 0:ln].rearrange(
                                        "p (h w) -> p h w", w=32),
                                    sc[1][:], 0.0, op0=ALU.add, op1=ALU.max)
                        else:
                            nc.scalar.activation(
                                _interior2(fm1, 0, 64, c0),
                                pt[0:64, 0:ln].rearrange(
                                    "p (h w) -> p h w", w=32),
                                AF.Relu, bias=sc[1][:], scale=1.0)
                    # staging for conv2: fm1 upper = fm1+1; scr = [fm1; fm1+34]
                    # (padded-span chunks, not packed ones)
                    for c0 in range(0, SPAN, 2048):
                        ln = min(2048, SPAN - c0)
                        a = G + c0
                        nc.sync.dma_start(out=fm1[64:128, a:a + ln],
                                          in_=fm1[0:64, a + 1:a + ln + 1])
                        nc.gpsimd.dma_start(out=scr[0:64, a:a + ln],
                                            in_=fm1[0:64, a:a + ln])
                        nc.gpsimd.dma_start(out=scr[64:128, a:a + ln],
                                            in_=fm1[0:64, a + 34:a + ln + 34])

                    # ---- conv2: 5 x K=128 per bank
                    for (c0, ln) in CH:
                        pt = cps.tile([128, 1024], F32, tag="cps", bufs=4)
                        for (bo, bl) in _banks(ln):
                            bank = (c0 + bo) // 512
                            mms = [(0, fm1, -35), (1, fm1, -1), (2, fm1, 33),
                                   (3, scr, -33), (4, scr, 35)]
                            for qi, (q, src, d) in enumerate(mms):
                                nc.tensor.matmul(
                                    pt[:, bo:bo + bl], c2w[:, q, :],
                                    _rhs(src, 0, 128, bank, d),
                                    start=(qi == 0), stop=(qi == 4))
                        if (c0 // 1024) % 2 == 0:
                            with nc.allow_low_precision(reason="bf16 fm"):
                                nc.vector.tensor_scalar(
                                    _interior2(fm2, 0, 128, c0),
                                    pt[:, 0:ln].rearrange("p (h w) -> p h w", w=32),
                                    sc[2][:], 0.0, op0=ALU.add, op1=ALU.max)
                        else:
                            nc.scalar.activation(
                                _interior2(fm2, 0, 128, c0),
                                pt[:, 0:ln].rearrange("p (h w) -> p h w", w=32),
                                AF.Relu, bias=sc[2][:], scale=1.0)

                    if k == 0:
                        rings(fm2, 128, nc.vector)
                    # ---- conv3 (fm3 lives in scr; conv2 reads of scr are done)
                    for (c0, ln) in CH:
                        pt = cps.tile([128, 1024], F32, tag="cps", bufs=4)
                        for (bo, bl) in _banks(ln):
                            bank = (c0 + bo) // 512
                            for t, (dy, dx) in enumerate(TAPS):
                                d = 34 * dy + dx
                                nc.tensor.matmul(
                                    pt[:, bo:bo + bl], c3w[:, t, :],
                                    _rhs(fm2, 0, 128, bank, d),
                                    start=(t == 0), stop=(t == 8))
                        if (c0 // 1024) % 2 == 1:
                            with nc.allow_low_precision(reason="bf16 fm"):
                                nc.vector.tensor_scalar(
                                    scr[:, G + c0:G + c0 + ln], pt[:, 0:ln],
                                    sc[3][:], 0.0, op0=ALU.add, op1=ALU.max)
                        else:
                            nc.scalar.activation(
                                scr[:, G + c0:G + c0 + ln], pt[:, 0:ln],
                                AF.Relu, bias=sc[3][:], scale=1.0)

                    # ---- width sums -> u (per frame, pipelined w/ conv3 drains)
                    for j in range(NI):
                        ivj = scr[:, G + j * 1024:G + (j + 1) * 1024].rearrange(
                            "p (h w) -> p h w", w=32)
                        with nc.allow_low_precision(reason="bf16 u; ~2e-3 ok"):
                            nc.vector.tensor_reduce(
                                u[:, b0 + j:b0 + j + 1, :], ivj,
                                axis=mybir.AxisListType.X, op=ALU.add)

                    # ---- incremental SSM for this sub-batch's 512 (b,tau) cols
                    ucols = u[:, b0:b0 + NI, :].rearrange("p a b -> p (a b)")
                    for m in range(2):
                        pm = cps.tile([128, 512], F32, tag="cps", bufs=4)
                        nc.tensor.matmul(pm[:], btw[:, 128 * m:128 * (m + 1)],
                                         ucols, start=True, stop=True)
                        tmp = big.tile([128, NI, L], F32, tag="tmp")
                        nc.vector.tensor_tensor(
                            tmp[:], pm[:].rearrange("p (a b) -> p a b", b=L),
                            wtw[:, m:m + 1, :].broadcast_to((128, NI, L)),
                            op=ALU.mult)
                        nc.vector.tensor_reduce(
                            sx[m][:, b0:b0 + NI], tmp[:],
                            axis=mybir.AxisListType.X, op=ALU.add)
                    with nc.allow_low_precision(reason="sum of bf16 u, f32 out"):
                        nc.vector.tensor_reduce(
                            ub[:, b0:b0 + NI], u[:, b0:b0 + NI, :],
                            axis=mybir.AxisListType.X, op=ALU.add)

                # ---- tail-only constants (emitted late: sync ring is idle now)
                nc.sync.dma_start(ctw[:], ct_d[:, :, :].rearrange("k p o -> p k o"))
                nc.sync.dma_start(dtw[:], dt_d[:, :])
                nc.sync.dma_start(w1w[:], w1_d[:, :, :].rearrange("m p o -> p m o"))
                nc.sync.dma_start(w2w[:], w2_d[:, :])
                nc.sync.dma_start(b1w[:], b1_d[:].unsqueeze(1))
                nc.sync.dma_start(b2w[0:10, :], b2_d[:].unsqueeze(1))
                nc.sync.dma_start(pbw[:], pb_d[:, :])
                make_identity(nc, ident)

            with tc.tile_pool(name="tail", bufs=1, space="PSUM") as tps:
                # pooled[o,b] = Ct.T@sx0 + Ct.T@sx1 + Dt.T@ub  (+ h0 bias)
                pooled_s = []
                o2s = big.tile([64, S], F32)
                for m in range(2):
                    pp = tps.tile([128, BL], F32, tag=f"pl{m}")
                    ops = [(ctw[:, 0, 128 * m:128 * (m + 1)], sx[0]),
                           (ctw[:, 1, 128 * m:128 * (m + 1)], sx[1]),
                           (dtw[:, 128 * m:128 * (m + 1)], ub)]
                    for i, (lt_, rt) in enumerate(ops):
                        nc.tensor.matmul(pp[:], lt_, rt[:],
                                         start=(i == 0), stop=(i == 2))
                    ps_t = big.tile([128, BL], F32, tag=f"pooled{m}")
                    nc.scalar.activation(ps_t[:], pp[:], AF.Identity,
                                         bias=pbw[:, m:m + 1], scale=1.0)
                    pooled_s.append(ps_t)
                    ptr = tps.tile([64, 128], F32, tag="ptr", bufs=2)
                    nc.tensor.transpose(ptr[:], ps_t[:], ident[:])
                    nc.vector.tensor_copy(o2s[:, 128 * m:128 * (m + 1)], ptr[:])
                nc.sync.dma_start(out2_d[:, :], o2s[:])

                # head
                hp = tps.tile([128, BL], F32, tag="hp")
                for m in range(2):
                    nc.tensor.matmul(hp[:], w1w[:, m, :], pooled_s[m][:],
                                     start=(m == 0), stop=(m == 1))
                hs = big.tile([128, BL], F32)
                nc.scalar.activation(hs[:], hp[:], AF.Relu, bias=b1w[:], scale=1.0)
                lp = tps.tile([16, BL], F32, tag="lp")
                nc.tensor.matmul(lp[0:10, :], w2w[:], hs[:], start=True, stop=True)
                ls = big.tile([16, BL], F32)
                nc.scalar.activation(ls[0:10, :], lp[0:10, :], AF.Identity,
                                     bias=b2w[0:10, :], scale=1.0)
                lt = tps.tile([64, 16], F32, tag="lt")
                nc.tensor.transpose(lt[:, 0:10], ls[0:10, :], ident[0:10, 0:10])
                o1s = big.tile([64, 16], F32)
                nc.vector.tensor_copy(o1s[:, 0:10], lt[:, 0:10])
                nc.sync.dma_start(out1_d[:, :], o1s[:, 0:10])

    nc.finalize()
    return nc


def prep_in_maps(inputs):
    f32 = np.float32
    bf = ml_dtypes.bfloat16

    scb0 = {}
    beta = {}
    for i in (1, 2, 3):
        g = np.asarray(inputs[f"bn{i}_g"], f32)
        b = np.asarray(inputs[f"bn{i}_b"], f32)
        m = np.asarray(inputs[f"bn{i}_m"], f32)
        v = np.asarray(inputs[f"bn{i}_v"], f32)
        inv = g / np.sqrt(v + np.float32(1e-5))
        scb0[i] = inv
        beta[i] = (b - m * inv).astype(f32)
    c1 = np.asarray(inputs["conv1_w"], dtype=f32) * scb0[1][:, None, None, None]
    c1T = np.zeros((32, 64), f32)
    for t, (dy, dx) in enumerate(TAPS):
        c1T[3 * t:3 * t + 3, :] = c1[:, :, dy + 1, dx + 1].T
    c2 = np.asarray(inputs["conv2_w"], dtype=f32) * scb0[2][:, None, None, None]
    c2q = np.zeros((5, 128, 128), f32)
    for i, dy in enumerate((-1, 0, 1)):
        c2q[i, 0:64, :] = c2[:, :, dy + 1, 0].T
        c2q[i, 64:128, :] = c2[:, :, dy + 1, 1].T
    c2q[3, 0:64, :] = c2[:, :, 0, 2].T
    c2q[3, 64:128, :] = c2[:, :, 1, 2].T
    c2q[4, 0:64, :] = c2[:, :, 2, 2].T
    c3 = np.asarray(inputs["conv3_w"], dtype=f32) * scb0[3][:, None, None, None]
    c3T = np.zeros((9, 128, 128), f32)
    for t, (dy, dx) in enumerate(TAPS):
        c3T[t] = c3[:, :, dy + 1, dx + 1].T

    A = -np.log1p(np.exp(np.asarray(inputs["ssm_A"], np.float64)))
    wts = np.stack([(1.0 - A ** (L - t)) / (1.0 - A) for t in range(L)], 1)  # (S,L)
    Wt = (wts / (32.0 * L)).astype(f32).reshape(2, 128, L).transpose(1, 0, 2)
    Wt = np.ascontiguousarray(Wt)
    BT = np.ascontiguousarray(np.asarray(inputs["ssm_B"], f32).T)
    Cm = np.asarray(inputs["ssm_C"], f32)
    Ct = np.ascontiguousarray(Cm.T.reshape(2, 128, S))
    Dt = np.ascontiguousarray((np.asarray(inputs["ssm_D"], np.float64).T / (32.0 * L)).astype(f32))
    h0 = np.asarray(inputs["ssm_h0"], np.float64)
    geo = A * (1.0 - A ** L) / (1.0 - A)
    pbias = ((Cm.astype(np.float64) @ (geo * h0)) / L).astype(f32).reshape(2, 128).T
    pbias = np.ascontiguousarray(pbias)

    w1T = np.ascontiguousarray(np.asarray(inputs["head_w1"], f32).T.reshape(2, 128, 128))
    w2T = np.ascontiguousarray(np.asarray(inputs["head_w2"], f32).T)
    hb1 = np.asarray(inputs["head_b1"], f32)
    hb2 = np.asarray(inputs["head_b2"], f32)

    shared = dict(c1T=c1T.astype(bf), c2q=c2q.astype(bf), c3T=c3T.astype(bf),
                  beta1=beta[1], beta2=beta[2], beta3=beta[3],
                  BT=BT.astype(bf), Wt=Wt, Ct=Ct, Dt=Dt, w1T=w1T, w2T=w2T,
                  hb1=hb1, hb2=hb2, pbias=pbias)

    x = np.asarray(inputs["x"], f32)
    in_maps = []
    for i in range(NCORES):
        xc = x[i * BL:(i + 1) * BL].reshape(SUB, NI, 3, 32, 32)
        wide = np.zeros((SUB, 3, W + 70), f32)
        wv = wide[:, :, 35 + G:35 + G + NI * FR].reshape(SUB, 3, NI, 34, 34)
        wv[:, :, :, 1:33, 1:33] = xc.transpose(0, 2, 1, 3, 4)
        xst = np.zeros((SUB, 32, W), f32)
        for t, (dy, dx) in enumerate(TAPS):
            d = 34 * dy + dx
            xst[:, 3 * t:3 * t + 3, :] = wide[:, :, 35 + d:35 + d + W]
        m = dict(shared)
        m["xst"] = np.ascontiguousarray(xst.astype(bf))
        in_maps.append(m)
    return in_maps


_NC_CACHE = []


def kernel(**inputs):
    if not _NC_CACHE:
        _NC_CACHE.append(build())
    nc = _NC_CACHE[0]
    in_maps = prep_in_maps(inputs)
    res = run_bass_kernel_spmd(nc, in_maps, core_ids=list(range(NCORES)))
    out = np.concatenate([res.results[i]["out1"] for i in range(NCORES)], axis=0)
    act = np.concatenate([res.results[i]["out2"] for i in range(NCORES)], axis=0)
    return out.astype(np.float32), act.astype(np.float32)


# revision 23
# speedup vs baseline: 1.0700x; 1.0391x over previous
"""Trainium2 Bass kernel for nn_CIFAR_SSM_Classifier.

Data-parallel over 8 NeuronCores: each core processes 64 of the 512 images.

Per-core pipeline (SBUF-resident, bf16 matmuls on the PE, fp32 accumulate).
All conv matmuls use the full K=128 contraction rows (partial-K matmuls let
the PE activity monitor drop the clock to 1.2 GHz):
  conv1 (3->64)   : 9 taps x 3ch stacked on K (27 rows, zero-padded to 128);
                    the tap-shifted/zero-padded input layout is staged host-side
                    and DMA'd once per sub-batch.
  conv2 (64->128) : 5 K=128 matmuls per 512-col bank: 3 pairs {(dy,-1),(dy,0)}
                    via fm1 upper half = fm1 shifted +1 col; 1 pair
                    {(-1,+1),(0,+1)} via scratch tile (lower=fm1, upper=fm1
                    shifted +34); 1 single (1,+1) with zero-padded weights.
  conv3 (128->128): 9 K=128 matmuls per bank.
  BN scale folded into conv weights host-side; drains are relu(psum+beta),
  split between the Vector and Scalar engines.
  Feature maps use a zero-padded 34x34 frame layout per image so all taps are
  plain column offsets of one SBUF tile.
  width-mean -> SSM: the L=32 diagonal scan is unrolled algebraically:
    sum_t x_t = sum_tau w_tau (.) (B u_tau),  w_tau = sum_{k<=L-1-tau} A^k
  and the Bu matmul + weighted tau-reduction run incrementally per sub-batch.
"""
import numpy as np
import ml_dtypes

import concourse.bass as bass
import concourse.tile as tile
from concourse import bacc, mybir
from concourse.bass_utils import run_bass_kernel_spmd
from concourse.masks import make_identity

F32 = mybir.dt.float32
BF16 = mybir.dt.bfloat16
AF = mybir.ActivationFunctionType
ALU = mybir.AluOpType

NCORES = 8
B = 512
BL = B // NCORES          # 64 images per core
NI = 16                   # images per sub-batch
SUB = BL // NI            # 4 sub-batches
FR = 34 * 34              # padded frame (34x34) per image
SPAN = NI * FR            # 18496
G = 72                    # guard columns on each side
W = G + SPAN + G
TAPS = [(dy, dx) for dy in (-1, 0, 1) for dx in (-1, 0, 1)]
PSPAN = NI * 1024          # packed interior span (matmul out / psum domain)
CH = [(c, 1024) for c in range(0, PSPAN, 1024)]
L = 32
S = 256


def _banks(length):
    return [(b, min(512, length - b)) for b in range(0, length, 512)]


def _rhs(src, p0, p1, bank, d):
    # interior pixels of half-frame `bank` (frame bank//2, rows 16*(bank%2)..+16)
    # of the padded layout, shifted by tap offset d
    n, hh = bank // 2, bank % 2
    base = G + n * FR + 35 + 544 * hh + d
    return src[p0:p1, base:base + 16 * 34].rearrange(
        "p (h w) -> p h w", w=34)[:, :, 0:32]


def _interior2(dst, p0, p1, c0):
    # interior pixels of frame c0//1024 of the padded layout (3-D view)
    n = c0 // 1024
    base = G + n * FR + 35
    return dst[p0:p1, base:base + 32 * 34].rearrange(
        "p (h w) -> p h w", w=34)[:, :, 0:32]


def build():
    nc = bacc.Bacc(None, target_bir_lowering=False, debug=False)

    x_d = nc.declare_dram_parameter("xst", [SUB, 32, W], BF16, isOutput=False)
    c1_d = nc.declare_dram_parameter("c1T", [32, 64], BF16, isOutput=False)
    c2q_d = nc.declare_dram_parameter("c2q", [5, 128, 128], BF16, isOutput=False)
    c3_d = nc.declare_dram_parameter("c3T", [9, 128, 128], BF16, isOutput=False)
    sc_d = {}
    for i, cc in ((1, 64), (2, 128), (3, 128)):
        sc_d[i] = nc.declare_dram_parameter(f"beta{i}", [cc], F32, isOutput=False)
    bt_d = nc.declare_dram_parameter("BT", [128, S], BF16, isOutput=False)
    wt_d = nc.declare_dram_parameter("Wt", [128, 2, L], F32, isOutput=False)
    ct_d = nc.declare_dram_parameter("Ct", [2, 128, S], F32, isOutput=False)
    dt_d = nc.declare_dram_parameter("Dt", [128, S], F32, isOutput=False)
    w1_d = nc.declare_dram_parameter("w1T", [2, 128, 128], F32, isOutput=False)
    w2_d = nc.declare_dram_parameter("w2T", [128, 10], F32, isOutput=False)
    b1_d = nc.declare_dram_parameter("hb1", [128], F32, isOutput=False)
    b2_d = nc.declare_dram_parameter("hb2", [10], F32, isOutput=False)
    pb_d = nc.declare_dram_parameter("pbias", [128, 2], F32, isOutput=False)
    out1_d = nc.declare_dram_parameter("out1", [BL, 10], F32, isOutput=True)
    out2_d = nc.declare_dram_parameter("out2", [BL, S], F32, isOutput=True)

    with tile.TileContext(nc) as tc:
        import contextlib
        with contextlib.ExitStack() as ctx:
            consts = ctx.enter_context(tc.tile_pool(name="consts", bufs=1))
            big = ctx.enter_context(tc.tile_pool(name="big", bufs=1))

            # ---- tiles
            c1w = consts.tile([32, 64], BF16)
            c2w = consts.tile([128, 5, 128], BF16)
            c3w = consts.tile([128, 9, 128], BF16)
            btw = consts.tile([128, S], BF16)
            wtw = consts.tile([128, 2, L], F32)
            ctw = consts.tile([128, 2, S], F32)
            dtw = consts.tile([128, S], F32)
            w1w = consts.tile([128, 2, 128], F32)
            w2w = consts.tile([128, 10], F32)
            b1w = consts.tile([128, 1], F32)
            b2w = consts.tile([16, 1], F32)
            pbw = consts.tile([128, 2], F32)
            ident = consts.tile([128, 128], F32)
            sc = {}
            for i, cc in ((1, 64), (2, 128), (3, 128)):
                sc[i] = consts.tile([cc, 1], F32, tag=f"beta{i}", name=f"beta{i}")

            x_st = big.tile([32, W], BF16)     # 27 stacked shifted taps of x
            fm1 = big.tile([128, W], BF16)     # 0-63: conv1 out; 64-127: +1 col
            fm2 = big.tile([128, W], BF16)
            scr = big.tile([128, W], BF16)     # conv2: fm1b (+0/+34); conv3: fm3
            u = big.tile([128, BL, L], BF16)   # width-sums, all 64 images
            sx = [big.tile([128, BL], F32, tag=f"sx{m}", name=f"sx{m}")
                  for m in range(2)]
            ub = big.tile([128, BL], F32)

            # ---- startup-critical loads; x[0] split across all 3 DMA rings
            # with column ranges ordered so conv1 consumes them just-in-time
            nc.sync.dma_start(c1w[:], c1_d[:, :])
            for i in (1, 2, 3):
                nc.sync.dma_start(sc[i][:], sc_d[i][:].unsqueeze(1))
            nc.scalar.dma_start(out=x_st[0:32, 4096:11264],
                                in_=x_d[0, :, 4096:11264])
            nc.gpsimd.dma_start(out=x_st[0:32, 11264:W],
                                in_=x_d[0, :, 11264:W])
            nc.sync.dma_start(out=x_st[0:32, 0:4096], in_=x_d[0, :, 0:4096])
            nc.sync.dma_start(c2w[:], c2q_d[:, :, :].rearrange("t k m -> k t m"))
            nc.sync.dma_start(c3w[:], c3_d[:, :, :].rearrange("t k m -> k t m"))
            # SSM weights used from sub-batch 0's tail
            nc.gpsimd.dma_start(btw[:], bt_d[:, :])
            nc.gpsimd.dma_start(wtw[:], wt_d[:, :, :])

            # minimal zero-init: x_st rows 27-127 (read with zero weights) and
            # the guard columns of the feature maps

            for t in (fm1, fm2, scr):
                nc.vector.memset(t[:, 0:G], 0.0)
                nc.vector.memset(t[:, G + SPAN:W], 0.0)

            rings_once = True

            def rings(t, p1, engine):  # noqa: E306
                for j in range(NI):
                    F0 = G + j * FR
                    engine.memset(t[0:p1, F0:F0 + 35], 0.0)
                    rb = t[0:p1, F0 + 67:F0 + 67 + 31 * 34].rearrange(
                        "p (a b) -> p a b", b=34)[:, :, 0:2]
                    engine.memset(rb, 0.0)
                    engine.memset(t[0:p1, F0 + 1121:F0 + 1156], 0.0)

            rings(fm1, 64, nc.vector)


            with tc.tile_pool(name="cps", bufs=2, space="PSUM") as cps:
                def conv1_chunk(ci, on_dve):
                    c0, ln = CH[ci]
                    pt = cps.tile([128, 1024], F32, tag="cps", bufs=4,
                                  name="c1pt")
                    for (bo, bl) in _banks(ln):
                        nc.tensor.matmul(
                            pt[0:64, bo:bo + bl], c1w[:],
                            _rhs(x_st, 0, 32, (c0 + bo) // 512, 0),
                            start=True, stop=True)
                    if on_dve:
                        with nc.allow_low_precision(reason="bf16 fm"):
                            nc.vector.tensor_scalar(
                                _interior2(fm1, 0, 64, c0),
                                pt[0:64, 0:ln].rearrange("p (h w) -> p h w", w=32),
                                sc[1][:], 0.0, op0=ALU.add, op1=ALU.max)
                    else:
                        nc.scalar.activation(
                            _interior2(fm1, 0, 64, c0),
                            pt[0:64, 0:ln].rearrange("p (h w) -> p h w", w=32),
                            AF.Relu, bias=sc[1][:], scale=1.0)

                for k in range(SUB):
                    b0 = k * NI
                    if k == 0:
                        for ci in range(len(CH)):
                            conv1_chunk(ci, on_dve=False)
                    # staging for conv2: fm1 upper = fm1+1; scr = [fm1; fm1+34]
                    # (padded-span chunks, not packed ones)
                    for c0 in range(0, SPAN, 2048):
                        ln = min(2048, SPAN - c0)
                        a = G + c0
                        nc.sync.dma_start(out=fm1[64:128, a:a + ln],
                                          in_=fm1[0:64, a + 1:a + ln + 1])
                        nc.gpsimd.dma_start(out=scr[0:64, a:a + ln],
                                            in_=fm1[0:64, a:a + ln])
                        nc.gpsimd.dma_start(out=scr[64:128, a:a + ln],
                                            in_=fm1[0:64, a + 34:a + ln + 34])

                    # ---- conv2: 5 x K=128 per bank
                    for (c0, ln) in CH:
                        pt = cps.tile([128, 1024], F32, tag="cps", bufs=4)
                        for (bo, bl) in _banks(ln):
                            bank = (c0 + bo) // 512
                            mms = [(0, fm1, -35), (1, fm1, -1), (2, fm1, 33),
                                   (3, scr, -33), (4, scr, 35)]
                            for qi, (q, src, d) in enumerate(mms):
                                nc.tensor.matmul(
                                    pt[:, bo:bo + bl], c2w[:, q, :],
                                    _rhs(src, 0, 128, bank, d),
                                    start=(qi == 0), stop=(qi == 4))
                        if (c0 // 1024) % 2 == 0:
                            with nc.allow_low_precision(reason="bf16 fm"):
                                nc.vector.tensor_scalar(
                                    _interior2(fm2, 0, 128, c0),
                                    pt[:, 0:ln].rearrange("p (h w) -> p h w", w=32),
                                    sc[2][:], 0.0, op0=ALU.add, op1=ALU.max)
                        else:
                            nc.scalar.activation(
                                _interior2(fm2, 0, 128, c0),
                                pt[:, 0:ln].rearrange("p (h w) -> p h w", w=32),
                                AF.Relu, bias=sc[2][:], scale=1.0)

                    if k == 0:
                        rings(fm2, 128, nc.vector)
                    if k + 1 < SUB:
                        for xj in range(4):
                            nc.sync.dma_start(
                                out=x_st[0:32, xj * 4660:min(W, 4660 * (xj + 1))],
                                in_=x_d[k + 1, :, xj * 4660:min(W, 4660 * (xj + 1))])
                    # ---- conv3 (fm3 lives in scr), with conv1[k+1] chunks
                    # interleaved so the K=32 matmuls sit between K=128 ones
                    for ci3, (c0, ln) in enumerate(CH):
                        if k + 1 < SUB and ci3 > 0:
                            conv1_chunk(ci3 - 1, on_dve=(ci3 % 2 == 0))
                        pt = cps.tile([128, 1024], F32, tag="cps", bufs=4)
                        for (bo, bl) in _banks(ln):
                            bank = (c0 + bo) // 512
                            for t, (dy, dx) in enumerate(TAPS):
                                d = 34 * dy + dx
                                nc.tensor.matmul(
                                    pt[:, bo:bo + bl], c3w[:, t, :],
                                    _rhs(fm2, 0, 128, bank, d),
                                    start=(t == 0), stop=(t == 8))
                        if (c0 // 1024) % 2 == 1:
                            with nc.allow_low_precision(reason="bf16 fm"):
                                nc.vector.tensor_scalar(
                                    scr[:, G + c0:G + c0 + ln], pt[:, 0:ln],
                                    sc[3][:], 0.0, op0=ALU.add, op1=ALU.max)
                        else:
                            nc.scalar.activation(
                                scr[:, G + c0:G + c0 + ln], pt[:, 0:ln],
                                AF.Relu, bias=sc[3][:], scale=1.0)

                    if k + 1 < SUB:
                        conv1_chunk(len(CH) - 1, on_dve=True)
                    # ---- width sums -> u (per frame, pipelined w/ conv3 drains)
                    for j in range(NI):
                        ivj = scr[:, G + j * 1024:G + (j + 1) * 1024].rearrange(
                            "p (h w) -> p h w", w=32)
                        with nc.allow_low_precision(reason="bf16 u; ~2e-3 ok"):
                            nc.vector.tensor_reduce(
                                u[:, b0 + j:b0 + j + 1, :], ivj,
                                axis=mybir.AxisListType.X, op=ALU.add)

                    # ---- incremental SSM for this sub-batch's 512 (b,tau) cols
                    ucols = u[:, b0:b0 + NI, :].rearrange("p a b -> p (a b)")
                    for m in range(2):
                        pm = cps.tile([128, 512], F32, tag="cps", bufs=4)
                        nc.tensor.matmul(pm[:], btw[:, 128 * m:128 * (m + 1)],
                                         ucols, start=True, stop=True)
                        tmp = big.tile([128, NI, L], F32, tag="tmp")
                        nc.vector.tensor_tensor(
                            tmp[:], pm[:].rearrange("p (a b) -> p a b", b=L),
                            wtw[:, m:m + 1, :].broadcast_to((128, NI, L)),
                            op=ALU.mult)
                        nc.vector.tensor_reduce(
                            sx[m][:, b0:b0 + NI], tmp[:],
                            axis=mybir.AxisListType.X, op=ALU.add)
                    with nc.allow_low_precision(reason="sum of bf16 u, f32 out"):
                        nc.vector.tensor_reduce(
                            ub[:, b0:b0 + NI], u[:, b0:b0 + NI, :],
                            axis=mybir.AxisListType.X, op=ALU.add)

                # ---- tail-only constants (emitted late: sync ring is idle now)
                nc.sync.dma_start(ctw[:], ct_d[:, :, :].rearrange("k p o -> p k o"))
                nc.sync.dma_start(dtw[:], dt_d[:, :])
                nc.sync.dma_start(w1w[:], w1_d[:, :, :].rearrange("m p o -> p m o"))
                nc.sync.dma_start(w2w[:], w2_d[:, :])
                nc.sync.dma_start(b1w[:], b1_d[:].unsqueeze(1))
                nc.sync.dma_start(b2w[0:10, :], b2_d[:].unsqueeze(1))
                nc.sync.dma_start(pbw[:], pb_d[:, :])
                make_identity(nc, ident)

            with tc.tile_pool(name="tail", bufs=1, space="PSUM") as tps:
                # pooled[o,b] = Ct.T@sx0 + Ct.T@sx1 + Dt.T@ub  (+ h0 bias)
                pooled_s = []
                o2s = big.tile([64, S], F32)
                for m in range(2):
                    pp = tps.tile([128, BL], F32, tag=f"pl{m}")
                    ops = [(ctw[:, 0, 128 * m:128 * (m + 1)], sx[0]),
                           (ctw[:, 1, 128 * m:128 * (m + 1)], sx[1]),
                           (dtw[:, 128 * m:128 * (m + 1)], ub)]
                    for i, (lt_, rt) in enumerate(ops):
                        nc.tensor.matmul(pp[:], lt_, rt[:],
                                         start=(i == 0), stop=(i == 2))
                    ps_t = big.tile([128, BL], F32, tag=f"pooled{m}")
                    nc.scalar.activation(ps_t[:], pp[:], AF.Identity,
                                         bias=pbw[:, m:m + 1], scale=1.0)
                    pooled_s.append(ps_t)
                    ptr = tps.tile([64, 128], F32, tag="ptr", bufs=2)
                    nc.tensor.transpose(ptr[:], ps_t[:], ident[:])
                    nc.vector.tensor_copy(o2s[:, 128 * m:128 * (m + 1)], ptr[:])
                nc.sync.dma_start(out2_d[:, :], o2s[:])

                # head
                hp = tps.tile([128, BL], F32, tag="hp")
                for m in range(2):
                    nc.tensor.matmul(hp[:], w1w[:, m, :], pooled_s[m][:],
                                     start=(m == 0), stop=(m == 1))
                hs = big.tile([128, BL], F32)
                nc.scalar.activation(hs[:], hp[:], AF.Relu, bias=b1w[:], scale=1.0)
                lp = tps.tile([16, BL], F32, tag="lp")
                nc.tensor.matmul(lp[0:10, :], w2w[:], hs[:], start=True, stop=True)
                ls = big.tile([16, BL], F32)
                nc.scalar.activation(ls[0:10, :], lp[0:10, :], AF.Identity,
                                     bias=b2w[0:10, :], scale=1.0)
                lt = tps.tile([64, 16], F32, tag="lt")
                nc.tensor.transpose(lt[:, 0:10], ls[0:10, :], ident[0:10, 0:10])
                o1s = big.tile([64, 16], F32)
                nc.vector.tensor_copy(o1s[:, 0:10], lt[:, 0:10])
                nc.sync.dma_start(out1_d[:, :], o1s[:, 0:10])

    nc.finalize()
    return nc


def prep_in_maps(inputs):
    f32 = np.float32
    bf = ml_dtypes.bfloat16

    scb0 = {}
    beta = {}
    for i in (1, 2, 3):
        g = np.asarray(inputs[f"bn{i}_g"], f32)
        b = np.asarray(inputs[f"bn{i}_b"], f32)
        m = np.asarray(inputs[f"bn{i}_m"], f32)
        v = np.asarray(inputs[f"bn{i}_v"], f32)
        inv = g / np.sqrt(v + np.float32(1e-5))
        scb0[i] = inv
        beta[i] = (b - m * inv).astype(f32)
    c1 = np.asarray(inputs["conv1_w"], dtype=f32) * scb0[1][:, None, None, None]
    c1T = np.zeros((32, 64), f32)
    for t, (dy, dx) in enumerate(TAPS):
        c1T[3 * t:3 * t + 3, :] = c1[:, :, dy + 1, dx + 1].T
    c2 = np.asarray(inputs["conv2_w"], dtype=f32) * scb0[2][:, None, None, None]
    c2q = np.zeros((5, 128, 128), f32)
    for i, dy in enumerate((-1, 0, 1)):
        c2q[i, 0:64, :] = c2[:, :, dy + 1, 0].T
        c2q[i, 64:128, :] = c2[:, :, dy + 1, 1].T
    c2q[3, 0:64, :] = c2[:, :, 0, 2].T
    c2q[3, 64:128, :] = c2[:, :, 1, 2].T
    c2q[4, 0:64, :] = c2[:, :, 2, 2].T
    c3 = np.asarray(inputs["conv3_w"], dtype=f32) * scb0[3][:, None, None, None]
    c3T = np.zeros((9, 128, 128), f32)
    for t, (dy, dx) in enumerate(TAPS):
        c3T[t] = c3[:, :, dy + 1, dx + 1].T

    A = -np.log1p(np.exp(np.asarray(inputs["ssm_A"], np.float64)))
    wts = np.stack([(1.0 - A ** (L - t)) / (1.0 - A) for t in range(L)], 1)  # (S,L)
    Wt = (wts / (32.0 * L)).astype(f32).reshape(2, 128, L).transpose(1, 0, 2)
    Wt = np.ascontiguousarray(Wt)
    BT = np.ascontiguousarray(np.asarray(inputs["ssm_B"], f32).T)
    Cm = np.asarray(inputs["ssm_C"], f32)
    Ct = np.ascontiguousarray(Cm.T.reshape(2, 128, S))
    Dt = np.ascontiguousarray((np.asarray(inputs["ssm_D"], np.float64).T / (32.0 * L)).astype(f32))
    h0 = np.asarray(inputs["ssm_h0"], np.float64)
    geo = A * (1.0 - A ** L) / (1.0 - A)
    pbias = ((Cm.astype(np.float64) @ (geo * h0)) / L).astype(f32).reshape(2, 128).T
    pbias = np.ascontiguousarray(pbias)

    w1T = np.ascontiguousarray(np.asarray(inputs["head_w1"], f32).T.reshape(2, 128, 128))
    w2T = np.ascontiguousarray(np.asarray(inputs["head_w2"], f32).T)
    hb1 = np.asarray(inputs["head_b1"], f32)
    hb2 = np.asarray(inputs["head_b2"], f32)

    shared = dict(c1T=c1T.astype(bf), c2q=c2q.astype(bf), c3T=c3T.astype(bf),
                  beta1=beta[1], beta2=beta[2], beta3=beta[3],
                  BT=BT.astype(bf), Wt=Wt, Ct=Ct, Dt=Dt, w1T=w1T, w2T=w2T,
                  hb1=hb1, hb2=hb2, pbias=pbias)

    x = np.asarray(inputs["x"], f32)
    in_maps = []
    for i in range(NCORES):
        xc = x[i * BL:(i + 1) * BL].reshape(SUB, NI, 3, 32, 32)
        wide = np.zeros((SUB, 3, W + 70), f32)
        wv = wide[:, :, 35 + G:35 + G + NI * FR].reshape(SUB, 3, NI, 34, 34)
        wv[:, :, :, 1:33, 1:33] = xc.transpose(0, 2, 1, 3, 4)
        xst = np.zeros((SUB, 32, W), f32)
        for t, (dy, dx) in enumerate(TAPS):
            d = 34 * dy + dx
            xst[:, 3 * t:3 * t + 3, :] = wide[:, :, 35 + d:35 + d + W]
        m = dict(shared)
        m["xst"] = np.ascontiguousarray(xst.astype(bf))
        in_maps.append(m)
    return in_maps


_NC_CACHE = []


def kernel(**inputs):
    if not _NC_CACHE:
        _NC_CACHE.append(build())
    nc = _NC_CACHE[0]
    in_maps = prep_in_maps(inputs)
    res = run_bass_kernel_spmd(nc, in_maps, core_ids=list(range(NCORES)))
    out = np.concatenate([res.results[i]["out1"] for i in range(NCORES)], axis=0)
    act = np.concatenate([res.results[i]["out2"] for i in range(NCORES)], axis=0)
    return out.astype(np.float32), act.astype(np.float32)


# revision 25
# speedup vs baseline: 1.0756x; 1.0052x over previous
"""Trainium2 Bass kernel for nn_CIFAR_SSM_Classifier.

Data-parallel over 8 NeuronCores: each core processes 64 of the 512 images.

Per-core pipeline (SBUF-resident, bf16 matmuls on the PE, fp32 accumulate).
All conv matmuls use the full K=128 contraction rows (partial-K matmuls let
the PE activity monitor drop the clock to 1.2 GHz):
  conv1 (3->64)   : 9 taps x 3ch stacked on K (27 rows, zero-padded to 128);
                    the tap-shifted/zero-padded input layout is staged host-side
                    and DMA'd once per sub-batch.
  conv2 (64->128) : 5 K=128 matmuls per 512-col bank: 3 pairs {(dy,-1),(dy,0)}
                    via fm1 upper half = fm1 shifted +1 col; 1 pair
                    {(-1,+1),(0,+1)} via scratch tile (lower=fm1, upper=fm1
                    shifted +34); 1 single (1,+1) with zero-padded weights.
  conv3 (128->128): 9 K=128 matmuls per bank.
  BN scale folded into conv weights host-side; drains are relu(psum+beta),
  split between the Vector and Scalar engines.
  Feature maps use a zero-padded 34x34 frame layout per image so all taps are
  plain column offsets of one SBUF tile.
  width-mean -> SSM: the L=32 diagonal scan is unrolled algebraically:
    sum_t x_t = sum_tau w_tau (.) (B u_tau),  w_tau = sum_{k<=L-1-tau} A^k
  and the Bu matmul + weighted tau-reduction run incrementally per sub-batch.
"""
import numpy as np
import ml_dtypes

import concourse.bass as bass
import concourse.tile as tile
from concourse import bacc, mybir
from concourse.bass_utils import run_bass_kernel_spmd
from concourse.masks import make_identity

F32 = mybir.dt.float32
BF16 = mybir.dt.bfloat16
AF = mybir.ActivationFunctionType
ALU = mybir.AluOpType

NCORES = 8
B = 512
BL = B // NCORES          # 64 images per core
NI = 16                   # images per sub-batch
SUB = BL // NI            # 4 sub-batches
FR = 34 * 34              # padded frame (34x34) per image
SPAN = NI * FR            # 18496
G = 72                    # guard columns on each side
W = G + SPAN + G
TAPS = [(dy, dx) for dy in (-1, 0, 1) for dx in (-1, 0, 1)]
PSPAN = NI * 1024          # packed interior span (matmul out / psum domain)
CH = [(c, 1024) for c in range(0, PSPAN, 1024)]
L = 32
S = 256


def _banks(length):
    return [(b, min(512, length - b)) for b in range(0, length, 512)]


def _rhs(src, p0, p1, bank, d):
    # interior pixels of half-frame `bank` (frame bank//2, rows 16*(bank%2)..+16)
    # of the padded layout, shifted by tap offset d
    n, hh = bank // 2, bank % 2
    base = G + n * FR + 35 + 544 * hh + d
    return src[p0:p1, base:base + 16 * 34].rearrange(
        "p (h w) -> p h w", w=34)[:, :, 0:32]


def _interior2(dst, p0, p1, c0):
    # interior pixels of frame c0//1024 of the padded layout (3-D view)
    n = c0 // 1024
    base = G + n * FR + 35
    return dst[p0:p1, base:base + 32 * 34].rearrange(
        "p (h w) -> p h w", w=34)[:, :, 0:32]


def build():
    nc = bacc.Bacc(None, target_bir_lowering=False, debug=False)

    x_d = nc.declare_dram_parameter("xst", [SUB, 32, W], BF16, isOutput=False)
    c1_d = nc.declare_dram_parameter("c1T", [32, 64], BF16, isOutput=False)
    c2q_d = nc.declare_dram_parameter("c2q", [5, 128, 128], BF16, isOutput=False)
    c3_d = nc.declare_dram_parameter("c3T", [9, 128, 128], BF16, isOutput=False)
    sc_d = {}
    for i, cc in ((1, 64), (2, 128), (3, 128)):
        sc_d[i] = nc.declare_dram_parameter(f"beta{i}", [cc], F32, isOutput=False)
    bt_d = nc.declare_dram_parameter("BT", [128, S], BF16, isOutput=False)
    wt_d = nc.declare_dram_parameter("Wt", [128, 2, L], F32, isOutput=False)
    ct_d = nc.declare_dram_parameter("Ct", [2, 128, S], F32, isOutput=False)
    dt_d = nc.declare_dram_parameter("Dt", [128, S], F32, isOutput=False)
    w1_d = nc.declare_dram_parameter("w1T", [2, 128, 128], F32, isOutput=False)
    w2_d = nc.declare_dram_parameter("w2T", [128, 10], F32, isOutput=False)
    b1_d = nc.declare_dram_parameter("hb1", [128], F32, isOutput=False)
    b2_d = nc.declare_dram_parameter("hb2", [10], F32, isOutput=False)
    pb_d = nc.declare_dram_parameter("pbias", [128, 2], F32, isOutput=False)
    out1_d = nc.declare_dram_parameter("out1", [BL, 10], F32, isOutput=True)
    out2_d = nc.declare_dram_parameter("out2", [BL, S], F32, isOutput=True)

    with tile.TileContext(nc) as tc:
        import contextlib
        with contextlib.ExitStack() as ctx:
            consts = ctx.enter_context(tc.tile_pool(name="consts", bufs=1))
            big = ctx.enter_context(tc.tile_pool(name="big", bufs=1))

            # ---- tiles
            c1w = consts.tile([32, 64], BF16)
            c2w = consts.tile([128, 5, 128], BF16)
            c3w = consts.tile([128, 9, 128], BF16)
            btw = consts.tile([128, S], BF16)
            wtw = consts.tile([128, 2, L], F32)
            ctw = consts.tile([128, 2, S], F32)
            dtw = consts.tile([128, S], F32)
            w1w = consts.tile([128, 2, 128], F32)
            w2w = consts.tile([128, 10], F32)
            b1w = consts.tile([128, 1], F32)
            b2w = consts.tile([16, 1], F32)
            pbw = consts.tile([128, 2], F32)
            ident = consts.tile([128, 128], F32)
            sc = {}
            for i, cc in ((1, 64), (2, 128), (3, 128)):
                sc[i] = consts.tile([cc, 1], F32, tag=f"beta{i}", name=f"beta{i}")

            x_st = big.tile([32, W], BF16)     # 27 stacked shifted taps of x
            fm1 = big.tile([128, W], BF16)     # 0-63: conv1 out; 64-127: +1 col
            fm2 = big.tile([128, W], BF16)
            scr = big.tile([128, W], BF16)     # conv2: fm1b (+0/+34); conv3: fm3
            u = big.tile([128, BL, L], BF16)   # width-sums, all 64 images
            sx = [big.tile([128, BL], F32, tag=f"sx{m}", name=f"sx{m}")
                  for m in range(2)]
            ub = big.tile([128, BL], F32)

            # ---- startup-critical loads; x[0] split across all 3 DMA rings
            # with column ranges ordered so conv1 consumes them just-in-time
            nc.sync.dma_start(c1w[:], c1_d[:, :])
            for i in (1, 2, 3):
                nc.sync.dma_start(sc[i][:], sc_d[i][:].unsqueeze(1))
            nc.scalar.dma_start(out=x_st[0:32, 4096:11264],
                                in_=x_d[0, :, 4096:11264])
            nc.gpsimd.dma_start(out=x_st[0:32, 11264:W],
                                in_=x_d[0, :, 11264:W])
            nc.sync.dma_start(out=x_st[0:32, 0:4096], in_=x_d[0, :, 0:4096])
            nc.sync.dma_start(c2w[:], c2q_d[:, :, :].rearrange("t k m -> k t m"))
            nc.sync.dma_start(c3w[:], c3_d[:, :, :].rearrange("t k m -> k t m"))
            # SSM weights used from sub-batch 0's tail
            nc.gpsimd.dma_start(btw[:], bt_d[:, :])
            nc.gpsimd.dma_start(wtw[:], wt_d[:, :, :])

            # minimal zero-init: x_st rows 27-127 (read with zero weights) and
            # the guard columns of the feature maps

            for t in (fm1, fm2, scr):
                nc.vector.memset(t[:, 0:G], 0.0)
                nc.vector.memset(t[:, G + SPAN:W], 0.0)

            rings_once = True

            def rings(t, p1, engine):  # noqa: E306
                for j in range(NI):
                    F0 = G + j * FR
                    engine.memset(t[0:p1, F0:F0 + 35], 0.0)
                    rb = t[0:p1, F0 + 67:F0 + 67 + 31 * 34].rearrange(
                        "p (a b) -> p a b", b=34)[:, :, 0:2]
                    engine.memset(rb, 0.0)
                    engine.memset(t[0:p1, F0 + 1121:F0 + 1156], 0.0)

            rings(fm1, 64, nc.vector)


            with tc.tile_pool(name="cps", bufs=2, space="PSUM") as cps:
                def conv1_chunk(ci, on_dve):
                    c0, ln = CH[ci]
                    pt = cps.tile([128, 1024], F32, tag="cps", bufs=4,
                                  name="c1pt")
                    for (bo, bl) in _banks(ln):
                        nc.tensor.matmul(
                            pt[0:64, bo:bo + bl], c1w[:],
                            _rhs(x_st, 0, 32, (c0 + bo) // 512, 0),
                            start=True, stop=True)
                    if on_dve:
                        with nc.allow_low_precision(reason="bf16 fm"):
                            nc.vector.tensor_scalar(
                                _interior2(fm1, 0, 64, c0),
                                pt[0:64, 0:ln].rearrange("p (h w) -> p h w", w=32),
                                sc[1][:], 0.0, op0=ALU.add, op1=ALU.max)
                    else:
                        nc.scalar.activation(
                            _interior2(fm1, 0, 64, c0),
                            pt[0:64, 0:ln].rearrange("p (h w) -> p h w", w=32),
                            AF.Relu, bias=sc[1][:], scale=1.0)

                warm = cps.tile([128, 1024], F32, tag="cps", bufs=4)
                for wi in range(22):
                    nc.tensor.matmul(warm[:, 0:512], fm1[:, 0:128],
                                     fm1[:, 128:640], start=(wi == 0),
                                     stop=(wi == 21))

                for k in range(SUB):
                    b0 = k * NI
                    if k == 0:
                        for ci in range(len(CH)):
                            conv1_chunk(ci, on_dve=False)
                    # scr staging (padded-span chunks); copyA too on k=0
                    for c0 in range(0, SPAN, 2048):
                        ln = min(2048, SPAN - c0)
                        a = G + c0
                        if k == 0:
                            nc.sync.dma_start(out=fm1[64:128, a:a + ln],
                                              in_=fm1[0:64, a + 1:a + ln + 1])
                        nc.gpsimd.dma_start(out=scr[0:64, a:a + ln],
                                            in_=fm1[0:64, a:a + ln])
                        nc.gpsimd.dma_start(out=scr[64:128, a:a + ln],
                                            in_=fm1[0:64, a + 34:a + ln + 34])

                    # ---- conv2: 5 x K=128 per bank
                    for (c0, ln) in CH:
                        pt = cps.tile([128, 1024], F32, tag="cps", bufs=4)
                        for (bo, bl) in _banks(ln):
                            bank = (c0 + bo) // 512
                            mms = [(0, fm1, -35), (1, fm1, -1), (2, fm1, 33),
                                   (3, scr, -33), (4, scr, 35)]
                            for qi, (q, src, d) in enumerate(mms):
                                nc.tensor.matmul(
                                    pt[:, bo:bo + bl], c2w[:, q, :],
                                    _rhs(src, 0, 128, bank, d),
                                    start=(qi == 0), stop=(qi == 4))
                        if (c0 // 1024) % 2 == 0:
                            with nc.allow_low_precision(reason="bf16 fm"):
                                nc.vector.tensor_scalar(
                                    _interior2(fm2, 0, 128, c0),
                                    pt[:, 0:ln].rearrange("p (h w) -> p h w", w=32),
                                    sc[2][:], 0.0, op0=ALU.add, op1=ALU.max)
                        else:
                            nc.scalar.activation(
                                _interior2(fm2, 0, 128, c0),
                                pt[:, 0:ln].rearrange("p (h w) -> p h w", w=32),
                                AF.Relu, bias=sc[2][:], scale=1.0)

                    if k == 0:
                        rings(fm2, 128, nc.vector)
                    if k + 1 < SUB:
                        for xj in range(4):
                            nc.sync.dma_start(
                                out=x_st[0:32, xj * 4660:min(W, 4660 * (xj + 1))],
                                in_=x_d[k + 1, :, xj * 4660:min(W, 4660 * (xj + 1))])
                    # ---- conv3 (fm3 lives in scr), with conv1[k+1] chunks
                    # interleaved so the K=32 matmuls sit between K=128 ones
                    def stage_chunk(j):
                        a = G + 2048 * j
                        ln2 = min(2048, SPAN - 2048 * j)
                        nc.sync.dma_start(out=fm1[64:128, a:a + ln2],
                                          in_=fm1[0:64, a + 1:a + ln2 + 1])

                    # staging chunk j (for k+1) is ready once conv1[k+1] has
                    # drained frame (2048*(j+1)+34)//1156
                    stage_after = {}
                    for j in range(9 + 1):
                        if 2048 * j < SPAN:
                            f = min(15, (2048 * (j + 1) + 34) // 1156)
                            stage_after.setdefault(f, []).append(j)
                    for ci3, (c0, ln) in enumerate(CH):
                        if k + 1 < SUB and ci3 > 0:
                            conv1_chunk(ci3 - 1, on_dve=(ci3 % 2 == 0))
                            for j in stage_after.get(ci3 - 1, []):
                                stage_chunk(j)
                        pt = cps.tile([128, 1024], F32, tag="cps", bufs=4)
                        for (bo, bl) in _banks(ln):
                            bank = (c0 + bo) // 512
                            for t, (dy, dx) in enumerate(TAPS):
                                d = 34 * dy + dx
                                nc.tensor.matmul(
                                    pt[:, bo:bo + bl], c3w[:, t, :],
                                    _rhs(fm2, 0, 128, bank, d),
                                    start=(t == 0), stop=(t == 8))
                        if (c0 // 1024) % 2 == 1:
                            with nc.allow_low_precision(reason="bf16 fm"):
                                nc.vector.tensor_scalar(
                                    scr[:, G + c0:G + c0 + ln], pt[:, 0:ln],
                                    sc[3][:], 0.0, op0=ALU.add, op1=ALU.max)
                        else:
                            nc.scalar.activation(
                                scr[:, G + c0:G + c0 + ln], pt[:, 0:ln],
                                AF.Relu, bias=sc[3][:], scale=1.0)

                    if k + 1 < SUB:
                        conv1_chunk(len(CH) - 1, on_dve=True)
                        for j in stage_after.get(15, []):
                            stage_chunk(j)
                    # ---- width sums -> u (per frame, pipelined w/ conv3 drains)
                    for j in range(NI):
                        ivj = scr[:, G + j * 1024:G + (j + 1) * 1024].rearrange(
                            "p (h w) -> p h w", w=32)
                        with nc.allow_low_precision(reason="bf16 u; ~2e-3 ok"):
                            nc.vector.tensor_reduce(
                                u[:, b0 + j:b0 + j + 1, :], ivj,
                                axis=mybir.AxisListType.X, op=ALU.add)

                    # ---- incremental SSM for this sub-batch's 512 (b,tau) cols
                    ucols = u[:, b0:b0 + NI, :].rearrange("p a b -> p (a b)")
                    for m in range(2):
                        pm = cps.tile([128, 512], F32, tag="cps", bufs=4)
                        nc.tensor.matmul(pm[:], btw[:, 128 * m:128 * (m + 1)],
                                         ucols, start=True, stop=True)
                        tmp = big.tile([128, NI, L], F32, tag="tmp")
                        nc.vector.tensor_tensor(
                            tmp[:], pm[:].rearrange("p (a b) -> p a b", b=L),
                            wtw[:, m:m + 1, :].broadcast_to((128, NI, L)),
                            op=ALU.mult)
                        nc.vector.tensor_reduce(
                            sx[m][:, b0:b0 + NI], tmp[:],
                            axis=mybir.AxisListType.X, op=ALU.add)
                    with nc.allow_low_precision(reason="sum of bf16 u, f32 out"):
                        nc.vector.tensor_reduce(
                            ub[:, b0:b0 + NI], u[:, b0:b0 + NI, :],
                            axis=mybir.AxisListType.X, op=ALU.add)

                # ---- tail-only constants (emitted late: sync ring is idle now)
                nc.sync.dma_start(ctw[:], ct_d[:, :, :].rearrange("k p o -> p k o"))
                nc.sync.dma_start(dtw[:], dt_d[:, :])
                nc.sync.dma_start(w1w[:], w1_d[:, :, :].rearrange("m p o -> p m o"))
                nc.sync.dma_start(w2w[:], w2_d[:, :])
                nc.sync.dma_start(b1w[:], b1_d[:].unsqueeze(1))
                nc.sync.dma_start(b2w[0:10, :], b2_d[:].unsqueeze(1))
                nc.sync.dma_start(pbw[:], pb_d[:, :])
                make_identity(nc, ident)

            with tc.tile_pool(name="tail", bufs=1, space="PSUM") as tps:
                # pooled[o,b] = Ct.T@sx0 + Ct.T@sx1 + Dt.T@ub  (+ h0 bias)
                pooled_s = []
                o2s = big.tile([64, S], F32)
                for m in range(2):
                    pp = tps.tile([128, BL], F32, tag=f"pl{m}")
                    ops = [(ctw[:, 0, 128 * m:128 * (m + 1)], sx[0]),
                           (ctw[:, 1, 128 * m:128 * (m + 1)], sx[1]),
                           (dtw[:, 128 * m:128 * (m + 1)], ub)]
                    for i, (lt_, rt) in enumerate(ops):
                        nc.tensor.matmul(pp[:], lt_, rt[:],
                                         start=(i == 0), stop=(i == 2))
                    ps_t = big.tile([128, BL], F32, tag=f"pooled{m}")
                    nc.scalar.activation(ps_t[:], pp[:], AF.Identity,
                                         bias=pbw[:, m:m + 1], scale=1.0)
                    pooled_s.append(ps_t)
                    ptr = tps.tile([64, 128], F32, tag="ptr", bufs=2)
                    nc.tensor.transpose(ptr[:], ps_t[:], ident[:])
                    nc.vector.tensor_copy(o2s[:, 128 * m:128 * (m + 1)], ptr[:])
                nc.sync.dma_start(out2_d[:, :], o2s[:])

                # head
                hp = tps.tile([128, BL], F32, tag="hp")
                for m in range(2):
                    nc.tensor.matmul(hp[:], w1w[:, m, :], pooled_s[m][:],
                                     start=(m == 0), stop=(m == 1))
                hs = big.tile([128, BL], F32)
                nc.scalar.activation(hs[:], hp[:], AF.Relu, bias=b1w[:], scale=1.0)
                lp = tps.tile([16, BL], F32, tag="lp")
                nc.tensor.matmul(lp[0:10, :], w2w[:], hs[:], start=True, stop=True)
                ls = big.tile([16, BL], F32)
                nc.scalar.activation(ls[0:10, :], lp[0:10, :], AF.Identity,
                                     bias=b2w[0:10, :], scale=1.0)
                lt = tps.tile([64, 16], F32, tag="lt")
                nc.tensor.transpose(lt[:, 0:10], ls[0:10, :], ident[0:10, 0:10])
                o1s = big.tile([64, 16], F32)
                nc.vector.tensor_copy(o1s[:, 0:10], lt[:, 0:10])
                nc.sync.dma_start(out1_d[:, :], o1s[:, 0:10])

    nc.finalize()
    return nc


def prep_in_maps(inputs):
    f32 = np.float32
    bf = ml_dtypes.bfloat16

    scb0 = {}
    beta = {}
    for i in (1, 2, 3):
        g = np.asarray(inputs[f"bn{i}_g"], f32)
        b = np.asarray(inputs[f"bn{i}_b"], f32)
        m = np.asarray(inputs[f"bn{i}_m"], f32)
        v = np.asarray(inputs[f"bn{i}_v"], f32)
        inv = g / np.sqrt(v + np.float32(1e-5))
        scb0[i] = inv
        beta[i] = (b - m * inv).astype(f32)
    c1 = np.asarray(inputs["conv1_w"], dtype=f32) * scb0[1][:, None, None, None]
    c1T = np.zeros((32, 64), f32)
    for t, (dy, dx) in enumerate(TAPS):
        c1T[3 * t:3 * t + 3, :] = c1[:, :, dy + 1, dx + 1].T
    c2 = np.asarray(inputs["conv2_w"], dtype=f32) * scb0[2][:, None, None, None]
    c2q = np.zeros((5, 128, 128), f32)
    for i, dy in enumerate((-1, 0, 1)):
        c2q[i, 0:64, :] = c2[:, :, dy + 1, 0].T
        c2q[i, 64:128, :] = c2[:, :, dy + 1, 1].T
    c2q[3, 0:64, :] = c2[:, :, 0, 2].T
    c2q[3, 64:128, :] = c2[:, :, 1, 2].T
    c2q[4, 0:64, :] = c2[:, :, 2, 2].T
    c3 = np.asarray(inputs["conv3_w"], dtype=f32) * scb0[3][:, None, None, None]
    c3T = np.zeros((9, 128, 128), f32)
    for t, (dy, dx) in enumerate(TAPS):
        c3T[t] = c3[:, :, dy + 1, dx + 1].T

    A = -np.log1p(np.exp(np.asarray(inputs["ssm_A"], np.float64)))
    wts = np.stack([(1.0 - A ** (L - t)) / (1.0 - A) for t in range(L)], 1)  # (S,L)
    Wt = (wts / (32.0 * L)).astype(f32).reshape(2, 128, L).transpose(1, 0, 2)
    Wt = np.ascontiguousarray(Wt)
    BT = np.ascontiguousarray(np.asarray(inputs["ssm_B"], f32).T)
    Cm = np.asarray(inputs["ssm_C"], f32)
    Ct = np.ascontiguousarray(Cm.T.reshape(2, 128, S))
    Dt = np.ascontiguousarray((np.asarray(inputs["ssm_D"], np.float64).T / (32.0 * L)).astype(f32))
    h0 = np.asarray(inputs["ssm_h0"], np.float64)
    geo = A * (1.0 - A ** L) / (1.0 - A)
    pbias = ((Cm.astype(np.float64) @ (geo * h0)) / L).astype(f32).reshape(2, 128).T
    pbias = np.ascontiguousarray(pbias)

    w1T = np.ascontiguousarray(np.asarray(inputs["head_w1"], f32).T.reshape(2, 128, 128))
    w2T = np.ascontiguousarray(np.asarray(inputs["head_w2"], f32).T)
    hb1 = np.asarray(inputs["head_b1"], f32)
    hb2 = np.asarray(inputs["head_b2"], f32)

    shared = dict(c1T=c1T.astype(bf), c2q=c2q.astype(bf), c3T=c3T.astype(bf),
                  beta1=beta[1], beta2=beta[2], beta3=beta[3],
                  BT=BT.astype(bf), Wt=Wt, Ct=Ct, Dt=Dt, w1T=w1T, w2T=w2T,
                  hb1=hb1, hb2=hb2, pbias=pbias)

    x = np.asarray(inputs["x"], f32)
    in_maps = []
    for i in range(NCORES):
        xc = x[i * BL:(i + 1) * BL].reshape(SUB, NI, 3, 32, 32)
        wide = np.zeros((SUB, 3, W + 70), f32)
        wv = wide[:, :, 35 + G:35 + G + NI * FR].reshape(SUB, 3, NI, 34, 34)
        wv[:, :, :, 1:33, 1:33] = xc.transpose(0, 2, 1, 3, 4)
        xst = np.zeros((SUB, 32, W), f32)
        for t, (dy, dx) in enumerate(TAPS):
            d = 34 * dy + dx
            xst[:, 3 * t:3 * t + 3, :] = wide[:, :, 35 + d:35 + d + W]
        m = dict(shared)
        m["xst"] = np.ascontiguousarray(xst.astype(bf))
        in_maps.append(m)
    return in_maps


_NC_CACHE = []


def kernel(**inputs):
    if not _NC_CACHE:
        _NC_CACHE.append(build())
    nc = _NC_CACHE[0]
    in_maps = prep_in_maps(inputs)
    res = run_bass_kernel_spmd(nc, in_maps, core_ids=list(range(NCORES)))
    out = np.concatenate([res.results[i]["out1"] for i in range(NCORES)], axis=0)
    act = np.concatenate([res.results[i]["out2"] for i in range(NCORES)], axis=0)
    return out.astype(np.float32), act.astype(np.float32)


# revision 26
# speedup vs baseline: 1.0787x; 1.0029x over previous
"""Trainium2 Bass kernel for nn_CIFAR_SSM_Classifier.

Data-parallel over 8 NeuronCores: each core processes 64 of the 512 images.

Per-core pipeline (SBUF-resident, bf16 matmuls on the PE, fp32 accumulate).
All conv matmuls use the full K=128 contraction rows (partial-K matmuls let
the PE activity monitor drop the clock to 1.2 GHz):
  conv1 (3->64)   : 9 taps x 3ch stacked on K (27 rows, zero-padded to 128);
                    the tap-shifted/zero-padded input layout is staged host-side
                    and DMA'd once per sub-batch.
  conv2 (64->128) : 5 K=128 matmuls per 512-col bank: 3 pairs {(dy,-1),(dy,0)}
                    via fm1 upper half = fm1 shifted +1 col; 1 pair
                    {(-1,+1),(0,+1)} via scratch tile (lower=fm1, upper=fm1
                    shifted +34); 1 single (1,+1) with zero-padded weights.
  conv3 (128->128): 9 K=128 matmuls per bank.
  BN scale folded into conv weights host-side; drains are relu(psum+beta),
  split between the Vector and Scalar engines.
  Feature maps use a zero-padded 34x34 frame layout per image so all taps are
  plain column offsets of one SBUF tile.
  width-mean -> SSM: the L=32 diagonal scan is unrolled algebraically:
    sum_t x_t = sum_tau w_tau (.) (B u_tau),  w_tau = sum_{k<=L-1-tau} A^k
  and the Bu matmul + weighted tau-reduction run incrementally per sub-batch.
"""
import numpy as np
import ml_dtypes

import concourse.bass as bass
import concourse.tile as tile
from concourse import bacc, mybir
from concourse.bass_utils import run_bass_kernel_spmd
from concourse.masks import make_identity

F32 = mybir.dt.float32
BF16 = mybir.dt.bfloat16
AF = mybir.ActivationFunctionType
ALU = mybir.AluOpType

NCORES = 8
B = 512
BL = B // NCORES          # 64 images per core
NI = 16                   # images per sub-batch
SUB = BL // NI            # 4 sub-batches
FR = 34 * 34              # padded frame (34x34) per image
SPAN = NI * FR            # 18496
G = 72                    # guard columns on each side
W = G + SPAN + G
TAPS = [(dy, dx) for dy in (-1, 0, 1) for dx in (-1, 0, 1)]
PSPAN = NI * 1024          # packed interior span (matmul out / psum domain)
CH = [(c, 1024) for c in range(0, PSPAN, 1024)]
L = 32
S = 256


def _banks(length):
    return [(b, min(512, length - b)) for b in range(0, length, 512)]


def _rhs(src, p0, p1, bank, d):
    # interior pixels of half-frame `bank` (frame bank//2, rows 16*(bank%2)..+16)
    # of the padded layout, shifted by tap offset d
    n, hh = bank // 2, bank % 2
    base = G + n * FR + 35 + 544 * hh + d
    return src[p0:p1, base:base + 16 * 34].rearrange(
        "p (h w) -> p h w", w=34)[:, :, 0:32]


def _interior2(dst, p0, p1, c0):
    # interior pixels of frame c0//1024 of the padded layout (3-D view)
    n = c0 // 1024
    base = G + n * FR + 35
    return dst[p0:p1, base:base + 32 * 34].rearrange(
        "p (h w) -> p h w", w=34)[:, :, 0:32]


def build():
    nc = bacc.Bacc(None, target_bir_lowering=False, debug=False)

    x_d = nc.declare_dram_parameter("xst", [SUB, 32, W], BF16, isOutput=False)
    c1_d = nc.declare_dram_parameter("c1T", [32, 64], BF16, isOutput=False)
    c2q_d = nc.declare_dram_parameter("c2q", [5, 128, 128], BF16, isOutput=False)
    c3_d = nc.declare_dram_parameter("c3T", [9, 128, 128], BF16, isOutput=False)
    sc_d = {}
    for i, cc in ((1, 64), (2, 128), (3, 128)):
        sc_d[i] = nc.declare_dram_parameter(f"beta{i}", [cc], F32, isOutput=False)
    bt_d = nc.declare_dram_parameter("BT", [128, S], BF16, isOutput=False)
    wt_d = nc.declare_dram_parameter("Wt", [128, 2, L], F32, isOutput=False)
    ct_d = nc.declare_dram_parameter("Ct", [2, 128, S], F32, isOutput=False)
    dt_d = nc.declare_dram_parameter("Dt", [128, S], F32, isOutput=False)
    w1_d = nc.declare_dram_parameter("w1T", [2, 128, 128], F32, isOutput=False)
    w2_d = nc.declare_dram_parameter("w2T", [128, 10], F32, isOutput=False)
    b1_d = nc.declare_dram_parameter("hb1", [128], F32, isOutput=False)
    b2_d = nc.declare_dram_parameter("hb2", [10], F32, isOutput=False)
    pb_d = nc.declare_dram_parameter("pbias", [128, 2], F32, isOutput=False)
    out1_d = nc.declare_dram_parameter("out1", [BL, 10], F32, isOutput=True)
    out2_d = nc.declare_dram_parameter("out2", [BL, S], F32, isOutput=True)

    with tile.TileContext(nc) as tc:
        import contextlib
        with contextlib.ExitStack() as ctx:
            consts = ctx.enter_context(tc.tile_pool(name="consts", bufs=1))
            big = ctx.enter_context(tc.tile_pool(name="big", bufs=1))

            # ---- tiles
            c1w = consts.tile([32, 64], BF16)
            c2w = consts.tile([128, 5, 128], BF16)
            c3w = consts.tile([128, 9, 128], BF16)
            btw = consts.tile([128, S], BF16)
            wtw = consts.tile([128, 2, L], F32)
            ctw = consts.tile([128, 2, S], F32)
            dtw = consts.tile([128, S], F32)
            w1w = consts.tile([128, 2, 128], F32)
            w2w = consts.tile([128, 10], F32)
            b1w = consts.tile([128, 1], F32)
            b2w = consts.tile([16, 1], F32)
            pbw = consts.tile([128, 2], F32)
            ident = consts.tile([128, 128], F32)
            sc = {}
            for i, cc in ((1, 64), (2, 128), (3, 128)):
                sc[i] = consts.tile([cc, 1], F32, tag=f"beta{i}", name=f"beta{i}")

            x_st = big.tile([32, W], BF16)     # 27 stacked shifted taps of x
            fm1 = big.tile([128, W], BF16)     # 0-63: conv1 out; 64-127: +1 col
            fm2 = big.tile([128, W], BF16)
            scr = big.tile([128, W], BF16)     # conv2: fm1b (+0/+34); conv3: fm3
            u = big.tile([128, BL, L], BF16)   # width-sums, all 64 images
            sx = [big.tile([128, BL], F32, tag=f"sx{m}", name=f"sx{m}")
                  for m in range(2)]
            ub = big.tile([128, BL], F32)

            # ---- startup-critical loads; x[0] split across all 3 DMA rings
            # with column ranges ordered so conv1 consumes them just-in-time
            nc.sync.dma_start(c1w[:], c1_d[:, :])
            for i in (1, 2, 3):
                nc.sync.dma_start(sc[i][:], sc_d[i][:].unsqueeze(1))
            nc.scalar.dma_start(out=x_st[0:32, 4096:11264],
                                in_=x_d[0, :, 4096:11264])
            nc.gpsimd.dma_start(out=x_st[0:32, 11264:W],
                                in_=x_d[0, :, 11264:W])
            nc.sync.dma_start(out=x_st[0:32, 0:4096], in_=x_d[0, :, 0:4096])
            nc.sync.dma_start(c2w[:], c2q_d[:, :, :].rearrange("t k m -> k t m"))
            nc.sync.dma_start(c3w[:], c3_d[:, :, :].rearrange("t k m -> k t m"))
            # SSM weights used from sub-batch 0's tail
            nc.gpsimd.dma_start(btw[:], bt_d[:, :])
            nc.gpsimd.dma_start(wtw[:], wt_d[:, :, :])

            # minimal zero-init: x_st rows 27-127 (read with zero weights) and
            # the guard columns of the feature maps

            for t in (fm1, fm2, scr):
                nc.vector.memset(t[:, 0:G], 0.0)
                nc.vector.memset(t[:, G + SPAN:W], 0.0)

            rings_once = True

            def rings(t, p1, engine):  # noqa: E306
                for j in range(NI):
                    F0 = G + j * FR
                    engine.memset(t[0:p1, F0:F0 + 35], 0.0)
                    rb = t[0:p1, F0 + 67:F0 + 67 + 31 * 34].rearrange(
                        "p (a b) -> p a b", b=34)[:, :, 0:2]
                    engine.memset(rb, 0.0)
                    engine.memset(t[0:p1, F0 + 1121:F0 + 1156], 0.0)

            rings(fm1, 64, nc.vector)


            with tc.tile_pool(name="cps", bufs=2, space="PSUM") as cps:
                def conv1_chunk(ci, on_dve):
                    c0, ln = CH[ci]
                    pt = cps.tile([128, 1024], F32, tag="cps", bufs=4,
                                  name="c1pt")
                    for (bo, bl) in _banks(ln):
                        nc.tensor.matmul(
                            pt[0:64, bo:bo + bl], c1w[:],
                            _rhs(x_st, 0, 32, (c0 + bo) // 512, 0),
                            start=True, stop=True)
                    if on_dve:
                        with nc.allow_low_precision(reason="bf16 fm"):
                            nc.vector.tensor_scalar(
                                _interior2(fm1, 0, 64, c0),
                                pt[0:64, 0:ln].rearrange("p (h w) -> p h w", w=32),
                                sc[1][:], 0.0, op0=ALU.add, op1=ALU.max)
                    else:
                        nc.scalar.activation(
                            _interior2(fm1, 0, 64, c0),
                            pt[0:64, 0:ln].rearrange("p (h w) -> p h w", w=32),
                            AF.Relu, bias=sc[1][:], scale=1.0)

                warm = cps.tile([128, 1024], F32, tag="cps", bufs=4)
                for wi in range(22):
                    nc.tensor.matmul(warm[:, 0:512], fm1[:, 0:128],
                                     fm1[:, 128:640], start=(wi == 0),
                                     stop=(wi == 21))

                for k in range(SUB):
                    b0 = k * NI
                    if k == 0:
                        for ci in range(len(CH)):
                            conv1_chunk(ci, on_dve=False)
                    if k == 0:
                        for c0 in range(0, SPAN, 2048):
                            ln = min(2048, SPAN - c0)
                            a = G + c0
                            nc.sync.dma_start(out=fm1[64:128, a:a + ln],
                                              in_=fm1[0:64, a + 1:a + ln + 1])
                            nc.gpsimd.dma_start(out=scr[0:64, a:a + ln],
                                                in_=fm1[0:64, a:a + ln])
                            nc.gpsimd.dma_start(out=scr[64:128, a:a + ln],
                                                in_=fm1[0:64, a + 34:a + ln + 34])

                    # ---- conv2: 5 x K=128 per bank
                    for (c0, ln) in CH:
                        pt = cps.tile([128, 1024], F32, tag="cps", bufs=4)
                        for (bo, bl) in _banks(ln):
                            bank = (c0 + bo) // 512
                            mms = [(0, fm1, -35), (1, fm1, -1), (2, fm1, 33),
                                   (3, scr, -33), (4, scr, 35)]
                            for qi, (q, src, d) in enumerate(mms):
                                nc.tensor.matmul(
                                    pt[:, bo:bo + bl], c2w[:, q, :],
                                    _rhs(src, 0, 128, bank, d),
                                    start=(qi == 0), stop=(qi == 4))
                        if (c0 // 1024) % 2 == 0:
                            with nc.allow_low_precision(reason="bf16 fm"):
                                nc.vector.tensor_scalar(
                                    _interior2(fm2, 0, 128, c0),
                                    pt[:, 0:ln].rearrange("p (h w) -> p h w", w=32),
                                    sc[2][:], 0.0, op0=ALU.add, op1=ALU.max)
                        else:
                            nc.scalar.activation(
                                _interior2(fm2, 0, 128, c0),
                                pt[:, 0:ln].rearrange("p (h w) -> p h w", w=32),
                                AF.Relu, bias=sc[2][:], scale=1.0)

                    if k == 0:
                        rings(fm2, 128, nc.vector)
                    if k + 1 < SUB:
                        for xj in range(4):
                            nc.sync.dma_start(
                                out=x_st[0:32, xj * 4660:min(W, 4660 * (xj + 1))],
                                in_=x_d[k + 1, :, xj * 4660:min(W, 4660 * (xj + 1))])
                    # ---- conv3 (fm3 lives in scr), with conv1[k+1] chunks
                    # interleaved so the K=32 matmuls sit between K=128 ones
                    def stage_chunk(j):
                        a = G + 2048 * j
                        ln2 = min(2048, SPAN - 2048 * j)
                        nc.sync.dma_start(out=fm1[64:128, a:a + ln2],
                                          in_=fm1[0:64, a + 1:a + ln2 + 1])

                    # staging chunk j (for k+1) is ready once conv1[k+1] has
                    # drained frame (2048*(j+1)+34)//1156
                    stage_after = {}
                    for j in range(9 + 1):
                        if 2048 * j < SPAN:
                            f = min(15, (2048 * (j + 1) + 34) // 1156)
                            stage_after.setdefault(f, []).append(j)
                    def u_reduce(f):
                        ivj = scr[:, G + f * 1024:G + (f + 1) * 1024].rearrange(
                            "p (h w) -> p h w", w=32)
                        with nc.allow_low_precision(reason="bf16 u; ~2e-3 ok"):
                            nc.vector.tensor_reduce(
                                u[:, b0 + f:b0 + f + 1, :], ivj,
                                axis=mybir.AxisListType.X, op=ALU.add)

                    def scr_stage(j):
                        a = G + 2048 * j
                        ln2 = min(2048, SPAN - 2048 * j)
                        nc.gpsimd.dma_start(out=scr[0:64, a:a + ln2],
                                            in_=fm1[0:64, a:a + ln2])
                        nc.gpsimd.dma_start(out=scr[64:128, a:a + ln2],
                                            in_=fm1[0:64, a + 34:a + ln2 + 34])

                    for ci3, (c0, ln) in enumerate(CH):
                        if ci3 > 0:
                            if k + 1 < SUB:
                                conv1_chunk(ci3 - 1, on_dve=(ci3 % 2 == 0))
                                for j in stage_after.get(ci3 - 1, []):
                                    stage_chunk(j)
                            u_reduce(ci3 - 1)
                            f = ci3 - 1
                            if k + 1 < SUB and f >= 2 and f % 2 == 0:
                                scr_stage((f - 2) // 2)
                        pt = cps.tile([128, 1024], F32, tag="cps", bufs=4)
                        for (bo, bl) in _banks(ln):
                            bank = (c0 + bo) // 512
                            for t, (dy, dx) in enumerate(TAPS):
                                d = 34 * dy + dx
                                nc.tensor.matmul(
                                    pt[:, bo:bo + bl], c3w[:, t, :],
                                    _rhs(fm2, 0, 128, bank, d),
                                    start=(t == 0), stop=(t == 8))
                        if (c0 // 1024) % 2 == 1:
                            with nc.allow_low_precision(reason="bf16 fm"):
                                nc.vector.tensor_scalar(
                                    scr[:, G + c0:G + c0 + ln], pt[:, 0:ln],
                                    sc[3][:], 0.0, op0=ALU.add, op1=ALU.max)
                        else:
                            nc.scalar.activation(
                                scr[:, G + c0:G + c0 + ln], pt[:, 0:ln],
                                AF.Relu, bias=sc[3][:], scale=1.0)

                    if k + 1 < SUB:
                        conv1_chunk(len(CH) - 1, on_dve=True)
                        for j in stage_after.get(15, []):
                            stage_chunk(j)
                    u_reduce(15)
                    if k + 1 < SUB:
                        scr_stage(6)
                        for j in (7, 8, 9):
                            scr_stage(j)

                    # ---- incremental SSM for this sub-batch's 512 (b,tau) cols
                    ucols = u[:, b0:b0 + NI, :].rearrange("p a b -> p (a b)")
                    for m in range(2):
                        pm = cps.tile([128, 512], F32, tag="cps", bufs=4)
                        nc.tensor.matmul(pm[:], btw[:, 128 * m:128 * (m + 1)],
                                         ucols, start=True, stop=True)
                        tmp = big.tile([128, NI, L], F32, tag="tmp")
                        nc.vector.tensor_tensor(
                            tmp[:], pm[:].rearrange("p (a b) -> p a b", b=L),
                            wtw[:, m:m + 1, :].broadcast_to((128, NI, L)),
                            op=ALU.mult)
                        nc.vector.tensor_reduce(
                            sx[m][:, b0:b0 + NI], tmp[:],
                            axis=mybir.AxisListType.X, op=ALU.add)
                    with nc.allow_low_precision(reason="sum of bf16 u, f32 out"):
                        nc.vector.tensor_reduce(
                            ub[:, b0:b0 + NI], u[:, b0:b0 + NI, :],
                            axis=mybir.AxisListType.X, op=ALU.add)

                # ---- tail-only constants (emitted late: sync ring is idle now)
                nc.sync.dma_start(ctw[:], ct_d[:, :, :].rearrange("k p o -> p k o"))
                nc.sync.dma_start(dtw[:], dt_d[:, :])
                nc.sync.dma_start(w1w[:], w1_d[:, :, :].rearrange("m p o -> p m o"))
                nc.sync.dma_start(w2w[:], w2_d[:, :])
                nc.sync.dma_start(b1w[:], b1_d[:].unsqueeze(1))
                nc.sync.dma_start(b2w[0:10, :], b2_d[:].unsqueeze(1))
                nc.sync.dma_start(pbw[:], pb_d[:, :])
                make_identity(nc, ident)

            with tc.tile_pool(name="tail", bufs=1, space="PSUM") as tps:
                # pooled[o,b] = Ct.T@sx0 + Ct.T@sx1 + Dt.T@ub  (+ h0 bias)
                pooled_s = []
                o2s = big.tile([64, S], F32)
                for m in range(2):
                    pp = tps.tile([128, BL], F32, tag=f"pl{m}")
                    ops = [(ctw[:, 0, 128 * m:128 * (m + 1)], sx[0]),
                           (ctw[:, 1, 128 * m:128 * (m + 1)], sx[1]),
                           (dtw[:, 128 * m:128 * (m + 1)], ub)]
                    for i, (lt_, rt) in enumerate(ops):
                        nc.tensor.matmul(pp[:], lt_, rt[:],
                                         start=(i == 0), stop=(i == 2))
                    ps_t = big.tile([128, BL], F32, tag=f"pooled{m}")
                    nc.scalar.activation(ps_t[:], pp[:], AF.Identity,
                                         bias=pbw[:, m:m + 1], scale=1.0)
                    pooled_s.append(ps_t)
                    ptr = tps.tile([64, 128], F32, tag="ptr", bufs=2)
                    nc.tensor.transpose(ptr[:], ps_t[:], ident[:])
                    nc.vector.tensor_copy(o2s[:, 128 * m:128 * (m + 1)], ptr[:])
                nc.sync.dma_start(out2_d[:, :], o2s[:])

                # head
                hp = tps.tile([128, BL], F32, tag="hp")
                for m in range(2):
                    nc.tensor.matmul(hp[:], w1w[:, m, :], pooled_s[m][:],
                                     start=(m == 0), stop=(m == 1))
                hs = big.tile([128, BL], F32)
                nc.scalar.activation(hs[:], hp[:], AF.Relu, bias=b1w[:], scale=1.0)
                lp = tps.tile([16, BL], F32, tag="lp")
                nc.tensor.matmul(lp[0:10, :], w2w[:], hs[:], start=True, stop=True)
                ls = big.tile([16, BL], F32)
                nc.scalar.activation(ls[0:10, :], lp[0:10, :], AF.Identity,
                                     bias=b2w[0:10, :], scale=1.0)
                lt = tps.tile([64, 16], F32, tag="lt")
                nc.tensor.transpose(lt[:, 0:10], ls[0:10, :], ident[0:10, 0:10])
                o1s = big.tile([64, 16], F32)
                nc.vector.tensor_copy(o1s[:, 0:10], lt[:, 0:10])
                nc.sync.dma_start(out1_d[:, :], o1s[:, 0:10])

    nc.finalize()
    return nc


def prep_in_maps(inputs):
    f32 = np.float32
    bf = ml_dtypes.bfloat16

    scb0 = {}
    beta = {}
    for i in (1, 2, 3):
        g = np.asarray(inputs[f"bn{i}_g"], f32)
        b = np.asarray(inputs[f"bn{i}_b"], f32)
        m = np.asarray(inputs[f"bn{i}_m"], f32)
        v = np.asarray(inputs[f"bn{i}_v"], f32)
        inv = g / np.sqrt(v + np.float32(1e-5))
        scb0[i] = inv
        beta[i] = (b - m * inv).astype(f32)
    c1 = np.asarray(inputs["conv1_w"], dtype=f32) * scb0[1][:, None, None, None]
    c1T = np.zeros((32, 64), f32)
    for t, (dy, dx) in enumerate(TAPS):
        c1T[3 * t:3 * t + 3, :] = c1[:, :, dy + 1, dx + 1].T
    c2 = np.asarray(inputs["conv2_w"], dtype=f32) * scb0[2][:, None, None, None]
    c2q = np.zeros((5, 128, 128), f32)
    for i, dy in enumerate((-1, 0, 1)):
        c2q[i, 0:64, :] = c2[:, :, dy + 1, 0].T
        c2q[i, 64:128, :] = c2[:, :, dy + 1, 1].T
    c2q[3, 0:64, :] = c2[:, :, 0, 2].T
    c2q[3, 64:128, :] = c2[:, :, 1, 2].T
    c2q[4, 0:64, :] = c2[:, :, 2, 2].T
    c3 = np.asarray(inputs["conv3_w"], dtype=f32) * scb0[3][:, None, None, None]
    c3T = np.zeros((9, 128, 128), f32)
    for t, (dy, dx) in enumerate(TAPS):
        c3T[t] = c3[:, :, dy + 1, dx + 1].T

    A = -np.log1p(np.exp(np.asarray(inputs["ssm_A"], np.float64)))
    wts = np.stack([(1.0 - A ** (L - t)) / (1.0 - A) for t in range(L)], 1)  # (S,L)
    Wt = (wts / (32.0 * L)).astype(f32).reshape(2, 128, L).transpose(1, 0, 2)
    Wt = np.ascontiguousarray(Wt)
    BT = np.ascontiguousarray(np.asarray(inputs["ssm_B"], f32).T)
    Cm = np.asarray(inputs["ssm_C"], f32)
    Ct = np.ascontiguousarray(Cm.T.reshape(2, 128, S))
    Dt = np.ascontiguousarray((np.asarray(inputs["ssm_D"], np.float64).T / (32.0 * L)).astype(f32))
    h0 = np.asarray(inputs["ssm_h0"], np.float64)
    geo = A * (1.0 - A ** L) / (1.0 - A)
    pbias = ((Cm.astype(np.float64) @ (geo * h0)) / L).astype(f32).reshape(2, 128).T
    pbias = np.ascontiguousarray(pbias)

    w1T = np.ascontiguousarray(np.asarray(inputs["head_w1"], f32).T.reshape(2, 128, 128))
    w2T = np.ascontiguousarray(np.asarray(inputs["head_w2"], f32).T)
    hb1 = np.asarray(inputs["head_b1"], f32)
    hb2 = np.asarray(inputs["head_b2"], f32)

    shared = dict(c1T=c1T.astype(bf), c2q=c2q.astype(bf), c3T=c3T.astype(bf),
                  beta1=beta[1], beta2=beta[2], beta3=beta[3],
                  BT=BT.astype(bf), Wt=Wt, Ct=Ct, Dt=Dt, w1T=w1T, w2T=w2T,
                  hb1=hb1, hb2=hb2, pbias=pbias)

    x = np.asarray(inputs["x"], f32)
    in_maps = []
    for i in range(NCORES):
        xc = x[i * BL:(i + 1) * BL].reshape(SUB, NI, 3, 32, 32)
        wide = np.zeros((SUB, 3, W + 70), f32)
        wv = wide[:, :, 35 + G:35 + G + NI * FR].reshape(SUB, 3, NI, 34, 34)
        wv[:, :, :, 1:33, 1:33] = xc.transpose(0, 2, 1, 3, 4)
        xst = np.zeros((SUB, 32, W), f32)
        for t, (dy, dx) in enumerate(TAPS):
            d = 34 * dy + dx
            xst[:, 3 * t:3 * t + 3, :] = wide[:, :, 35 + d:35 + d + W]
        m = dict(shared)
        m["xst"] = np.ascontiguousarray(xst.astype(bf))
        in_maps.append(m)
    return in_maps


_NC_CACHE = []


def kernel(**inputs):
    if not _NC_CACHE:
        _NC_CACHE.append(build())
    nc = _NC_CACHE[0]
    in_maps = prep_in_maps(inputs)
    res = run_bass_kernel_spmd(nc, in_maps, core_ids=list(range(NCORES)))
    out = np.concatenate([res.results[i]["out1"] for i in range(NCORES)], axis=0)
    act = np.concatenate([res.results[i]["out2"] for i in range(NCORES)], axis=0)
    return out.astype(np.float32), act.astype(np.float32)
